# revision 49
# baseline (speedup 1.0000x reference)
"""OHEM cross-entropy loss (CriterionOhem) on 8 Trainium2 NeuronCores.

Reference semantics (N = 4*512*1024 pixels, C = 19 classes):
  p_i     = softmax(pred)[i, t_i]                (true-class prob per pixel)
  kth     = sort(p)[MIN_KEPT-1]
  thr     = max(kth, 0.7)
  keep_i  = p_i <= thr
  loss    = sum(keep_i * nll_i) / max(count(keep), 1)

Key reduction: if count(p <= 0.7) >= MIN_KEPT then kth <= 0.7 and thr == 0.7
exactly, so no top-k is needed — only a masked sum + count, which the host
verifies from the device partials (falling back to a numpy reference in the
degenerate case, which cannot occur for randn logits).

Host-side trick: per pixel, the target-class logit is swapped into class
slot 0 (a pure permutation — the softmax denominator is permutation
invariant and x_t becomes the slot-0 plane).  This removes every trace of
the target from the device program: no replicated (t - c) bytes on the
wire, no masked-logit pass on DVE, no second matmul pass on PE.

Device layout per core (262,144 pixels = 128 blocks x 2048):
  xk   [128*19, 2048] fp8    class-rows, row g = 19*block + class
  xk16 [3*128, 2048]  fp16   the three DVE_TILES' rows (DVE needs 2-byte)
  x0   [128, 2048]    fp16   slot-0 (target-class) logits + NLL_MIN
  19 tiles of 128 consecutive class-rows stream through:
    ACT: et = exp(x) for 16 tiles (chunked to amortize fixed overhead);
    DVE: et = exp(x) for 3 tiles via a bit-trick exp2 (engine balancing);
    PE : one-hot block maps contract the 19 classes of each block into
         S[block, pixel], accumulated across all 19 tiles into four
         [128, 512] fp32 PSUM banks (start@k=0 / stop@k=18).
  Finals per bank as its accumulation group stops: lnS = ln(S) on ACT;
  d1 = lnS - x0' on DVE; rl = relu(d1), kp = (d1 >= 0); PE ones-matmuls
  fold partitions into R[1, 512] accumulators across banks; host sums
  num = sum(rl) + NLL_MIN*cnt, cnt = sum(kp).
"""

import numpy as np
from contextlib import ExitStack

import concourse.bass as bass
import concourse.tile as tile
from concourse import bacc
from concourse import mybir
from concourse.bass_utils import run_bass_kernel_spmd

F32 = mybir.dt.float32
F16 = mybir.dt.float16
F8 = mybir.dt.float8e4
AF = mybir.ActivationFunctionType
OP = mybir.AluOpType

C = 19
THRESH = 0.7
MIN_KEPT = 100000
NLL_MIN = float(-np.log(np.float32(THRESH)))  # keep <=> nll >= -ln(0.7)

# Full-size geometry: 4x19x512x1024 pred over 8 cores.
BATCH, HH, WW = 4, 512, 1024
NCORES = 8


class Geo:
    def __init__(self, X, nblk):
        self.X = X                      # pixels per block (free axis)
        self.NBLK = nblk                # blocks per core (psum partitions)
        self.NROWS = C * nblk           # class-rows per core
        self.NT = self.NROWS // 128     # 128-row tiles
        assert self.NROWS % 128 == 0
        self.NPIX = nblk * X            # pixels per core


GEO_FULL = Geo(2048, 128)               # 128 x 2048 = 262,144 pixels/core


def make_consts(g):
    """Per-tile one-hot block maps, packed side by side: map k column block
    [k*128,(k+1)*128) has a one at (p, (128k+p)//19)."""
    maps = np.zeros((128, g.NT * 128), np.float16)
    for k in range(g.NT):
        for p in range(128):
            maps[p, k * 128 + (128 * k + p) // C] = 1.0
    return {"maps": maps}


ACT_TABLE_LN_EXP = 6  # natural_log_exp_and_others in act_info.json

# Tiles whose exp runs on DVE (bit-trick exp2) instead of ACT, balancing the
# two engines; spread through the stream so matmul feeding stays smooth.
DVE_TILES = (4, 9, 14)
# Minimax quadratic for 2^f on [-0.5, 0.5]: 2^f ~= S2*(f+A2)^2 + C2.
S2, A2, C2 = 0.22266791031510733, 1.556176036576733, 0.4609102972256174
L2E = float(np.log2(np.e))


def emit(ctx, tc, g, xk, xk16, x0, maps, acc):
    nc = tc.nc
    X = g.X
    NSLC = X // 512
    I16 = mybir.dt.int16

    # ACT work items: chunks of tiles sharing one SBUF tile / ACT exp.
    # Singles at both ends (fast first exp; short final matmul backlog
    # behind the last exp); pairs in the middle amortize the fixed
    # per-instruction overhead.  DVE tiles are woven in between.
    act_tiles = [k for k in range(g.NT) if k not in DVE_TILES]
    chunks = [(act_tiles[0],)]
    i = 1
    while i + 2 < len(act_tiles):
        chunks.append((act_tiles[i], act_tiles[i + 1]))
        i += 2
    chunks += [(k,) for k in act_tiles[i:]]
    # DVE exp work is woven in EARLY (DVE is otherwise idle until the
    # finals), but its matmuls are deferred to the very end: PE executes
    # in order, so a DVE-gated matmul mid-stream would block all later
    # ACT-tile matmuls.  The accumulation stop therefore sits on the last
    # DVE tile.
    items = []
    for ci, ch in enumerate(chunks):
        items.append(("act", ch))
        if 0 < ci <= len(DVE_TILES):
            items.append(("dve", ci - 1))

    xvp = ctx.enter_context(tc.tile_pool(name="xvp", bufs=3))
    etp = ctx.enter_context(tc.tile_pool(name="etp", bufs=3))
    cst = ctx.enter_context(tc.tile_pool(name="cst", bufs=1))
    pss = ctx.enter_context(tc.tile_pool(name="pss", bufs=1, space="PSUM"))
    fin = ctx.enter_context(tc.tile_pool(name="fin", bufs=1))
    lnp = ctx.enter_context(tc.tile_pool(name="lnp", bufs=2))
    rkp = ctx.enter_context(tc.tile_pool(name="rkp", bufs=2))
    dxp = ctx.enter_context(tc.tile_pool(name="dxp", bufs=2))
    dep = ctx.enter_context(tc.tile_pool(name="dep", bufs=len(DVE_TILES)))
    dsp = ctx.enter_context(tc.tile_pool(name="dsp", bufs=2))

    # One activation table serves both Exp and Ln; load it up front so the
    # compiler's table-load pass inserts nothing mid-stream.
    nc.scalar.add_instruction(mybir.InstLoadActFuncSet(
        name=nc.get_next_instruction_name(), ins=[], outs=[],
        act_func_set_id=ACT_TABLE_LN_EXP))

    maps_t = cst.tile([128, g.NT * 128], F16)
    x0t = cst.tile([128, X], F16)
    ones_t = cst.tile([128, 1], F16)
    nc.vector.memset(ones_t[:], 1.0)

    # One PSUM tile per 512-column bank: a PSUM reader waits for its whole
    # accumulation group, so separate tiles let each bank's finals start as
    # soon as that bank's own 19-matmul group stops.
    S_ps = [pss.tile([128, 512], F32, name=f"S{m}") for m in range(NSLC)]
    # Final partial sums: R[0,:] accumulates relu(d1) column sums, R[1,:]
    # keep-count column sums, via ones-vector matmuls over the 4 banks.
    R_rl = pss.tile([1, 512], F32)
    R_kp = pss.tile([1, 512], F32)

    def mm_tile(k, et, off, m0=0, m1=NSLC):
        lhs = maps_t[:, k * 128:(k + 1) * 128]
        for m in range(m0, m1):
            nc.tensor.matmul(S_ps[m][:, :], lhs,
                             et[:, off + m * 512:off + (m + 1) * 512],
                             start=(k == 0), stop=(k == g.NT - 1))

    def dma_item(i):
        kind, v = items[i]
        if kind == "act":
            xv = xvp.tile([128, len(v) * X], F8, tag="x")
            for j, k in enumerate(v):
                nc.sync.dma_start(xv[:, j * X:(j + 1) * X],
                                  xk[128 * k:128 * (k + 1), :])
        else:
            xv = dxp.tile([128, X], F16, tag="dx")
            nc.sync.dma_start(xv[:], xk16[128 * v:128 * (v + 1), :])
        # Map loads: small per-chunk slices early (per-DMA HWDGE setup is
        # ~625ns, but a single bulk load early would displace the pixel
        # tiles that gate ACT), then one bulk DMA for tiles 8+ once the
        # queue has slack.  Each lands before the matmuls that read it.
        if i == 0:
            nc.sync.dma_start(maps_t[:, 0:128], maps[:, 0:128])
        elif i == 1:
            nc.sync.dma_start(maps_t[:, 128:384], maps[:, 128:384])
        elif i == 3:
            nc.sync.dma_start(maps_t[:, 384:768], maps[:, 384:768])
        elif i == 5:
            nc.sync.dma_start(maps_t[:, 768:1024], maps[:, 768:1024])
        elif i == 6:
            nc.sync.dma_start(maps_t[:, 1024:], maps[:, 1024:])
        if i == len(items) - 2:
            nc.sync.dma_start(x0t[:], x0)
        return xv

    def dve_exp(xv, et):
        """et = exp(xv) on DVE: 2^(x*log2 e) via the fp16 +1536 rounding
        trick for the integer part (exponent-field construction) and a
        minimax quadratic for 2^frac.  ~0.9% max rel error."""
        s1 = dsp.tile([128, X], F16, tag="s1")
        s2 = dsp.tile([128, X], F16, tag="s2")
        s3 = dsp.tile([128, X], F16, tag="s3")
        TS, TT = nc.vector.tensor_scalar, nc.vector.tensor_tensor
        TS(out=s1[:], in0=xv[:], scalar1=L2E, scalar2=1536.0,
           op0=OP.mult, op1=OP.add)                      # s1 = 1536 + n
        TS(out=s2[:], in0=s1[:], scalar1=-1536.0, scalar2=None,
           op0=OP.add)                                   # s2 = n
        TS(out=s3[:], in0=xv[:], scalar1=L2E, scalar2=None,
           op0=OP.mult)                                  # s3 = y
        TT(out=s3[:], in0=s3[:], in1=s2[:], op=OP.subtract)  # s3 = f
        TS(out=s1[:].bitcast(I16), in0=s1[:].bitcast(I16),
           scalar1=15 - 0x6600, scalar2=None, op0=OP.add)
        TS(out=s1[:].bitcast(I16), in0=s1[:].bitcast(I16),
           scalar1=10, scalar2=None, op0=OP.logical_shift_left)  # s1 = 2^n
        TS(out=s2[:], in0=s3[:], scalar1=float(A2), scalar2=None,
           op0=OP.add)                                   # s2 = f + A2
        TT(out=s2[:], in0=s2[:], in1=s2[:], op=OP.mult)  # s2 = (f+A2)^2
        TS(out=s2[:], in0=s2[:], scalar1=float(S2), scalar2=float(C2),
           op0=OP.mult, op1=OP.add)                      # s2 ~= 2^f
        TT(out=et[:], in0=s2[:], in1=s1[:], op=OP.mult)  # et = 2^f * 2^n

    # Software-pipelined: item i+1's DMAs are emitted (and queued) before
    # item i's compute so the load stream never waits on compute emission.
    dve_ets = []
    xv_next = dma_item(0)
    for i, (kind, v) in enumerate(items):
        xv = xv_next
        if i + 1 < len(items):
            xv_next = dma_item(i + 1)
        if kind == "dve":
            et = dep.tile([128, X], F16, tag="de")
            dve_exp(xv, et)
            dve_ets.append(et)
            continue
        if i == len(items) - 2:
            # Deferred DVE-tile matmuls (their ets completed mid-stream):
            # emitted here so PE digests them under the second-to-last
            # exp, leaving only the last tile's matmuls trailing.
            for m in range(NSLC):
                for di, det in enumerate(dve_ets):
                    mm_tile(DVE_TILES[di], det, 0, m, m + 1)
        W = len(v) * X
        et = etp.tile([128, W], F16, tag="e")
        # The last tile's exp runs in halves so its matmuls (which gate the
        # finals) start two PSUM banks early.
        hsplit = 2 if i == len(items) - 1 else 1
        for h in range(hsplit):
            hw = W // hsplit
            nc.scalar.activation(et[:, h * hw:(h + 1) * hw],
                                 xv[:, h * hw:(h + 1) * hw], AF.Exp)
            for j, k in enumerate(v):
                mm_tile(k, et, j * X, h * NSLC // hsplit,
                        (h + 1) * NSLC // hsplit)

    # Per-bank finals (x0 already holds x_t + NLL_MIN from the host):
    #   d1 = ln S - x0' ; rl = relu(d1) ; kp = (d1 >= 0)
    # then PE ones-matmuls fold the partition dimension into R_{rl,kp},
    # accumulating across banks; the host sums the 512-wide rows.
    for m in range(NSLC):
        sl = slice(m * 512, (m + 1) * 512)
        lnS = lnp.tile([128, 512], F16, tag="ln")
        nc.scalar.activation(lnS[:], S_ps[m][:, :], AF.Ln)
        rk = rkp.tile([128, 3, 512], F16, tag="rk")
        d1 = rk[:, 2, :]
        nc.vector.tensor_sub(d1, lnS[:], x0t[:, sl])
        nc.vector.tensor_scalar(out=rk[:, 0, :], in0=d1, scalar1=0.0,
                                scalar2=None, op0=OP.max)
        nc.vector.tensor_scalar(out=rk[:, 1, :], in0=d1, scalar1=0.0,
                                scalar2=None, op0=OP.is_ge)
        nc.tensor.matmul(R_rl[:, :], ones_t[:], rk[:, 0, :],
                         start=(m == 0), stop=(m == NSLC - 1))
        nc.tensor.matmul(R_kp[:, :], ones_t[:], rk[:, 1, :],
                         start=(m == 0), stop=(m == NSLC - 1))

    # Evacuate the two PSUM rows in parallel on ACT and DVE (both engines
    # must address partition 0 — engine partition bases are 0/32/64/96),
    # then one DMA.
    accs = fin.tile([1, 1024], F32)
    nc.scalar.copy(accs[0:1, 0:512], R_rl[:, :])
    nc.vector.tensor_scalar(out=accs[0:1, 512:1024], in0=R_kp[:, :],
                            scalar1=0.0, scalar2=None, op0=OP.add)
    nc.sync.dma_start(acc[:, :], accs[:])


def build_nc(g):
    nc = bacc.Bacc(
        "TRN2",
        target_bir_lowering=False,
        debug=False,
        enable_asserts=True,
        num_devices=NCORES,
    )
    xk = nc.dram_tensor("xk", [g.NROWS, g.X], F8, kind="ExternalInput")
    xk16 = nc.dram_tensor("xk16", [len(DVE_TILES) * 128, g.X], F16,
                          kind="ExternalInput")
    x0 = nc.dram_tensor("x0", [g.NBLK, g.X], F16, kind="ExternalInput")
    maps = nc.dram_tensor("maps", [128, g.NT * 128], F16, kind="ExternalInput")
    acc = nc.dram_tensor("acc", [1, 1024], F32, kind="ExternalOutput")
    with tile.TileContext(nc) as tc, ExitStack() as ctx:
        emit(ctx, tc, g, xk.ap(), xk16.ap(), x0.ap(), maps.ap(), acc.ap())
    nc.compile()
    return nc


_NC_CACHE = {}


def _get_nc(g):
    key = (g.X, g.NBLK)
    if key not in _NC_CACHE:
        _NC_CACHE[key] = build_nc(g)
    return _NC_CACHE[key]


def make_inputs(pred_slice, target_slice, g):
    """Per-core packed inputs.  xk[b*19 + c, px] holds the fp8 logits with
    the target class swapped into slot 0; x0 is the slot-0 (target-class)
    fp16 plane with the keep threshold pre-added (device computes
    d1 = ln S - x0')."""
    import ml_dtypes
    xk = np.empty((g.NBLK, C, g.X), np.float16)
    xk[:] = pred_slice.reshape(C, g.NBLK, g.X).swapaxes(0, 1)
    t = target_slice.reshape(g.NBLK, g.X).astype(np.intp)
    bi = np.arange(g.NBLK)[:, None]
    ci = np.arange(g.X)[None, :]
    xt = xk[bi, t, ci].copy()
    xk[bi, t, ci] = xk[:, 0, :]
    xk[:, 0, :] = xt
    xkr = xk.reshape(g.NROWS, g.X)
    xk8 = xkr.astype(ml_dtypes.float8_e4m3)
    xk16 = np.concatenate([xkr[128 * k:128 * (k + 1), :] for k in DVE_TILES])
    x0 = (xt.astype(np.float32) + np.float32(NLL_MIN)).astype(np.float16)
    return {"xk": xk8, "xk16": xk16, "x0": x0}


def _shard_inputs(pred, target, g):
    """Slice the full inputs into per-core in_maps (8 cores)."""
    consts = make_consts(g)
    in_maps = []
    rows_per_core = HH // 2  # 256
    for core in range(NCORES):
        b, half = core // 2, core % 2
        h0 = half * rows_per_core
        m = make_inputs(pred[b, :, h0:h0 + rows_per_core, :],
                        target[b, h0:h0 + rows_per_core, :], g)
        m.update(consts)
        in_maps.append(m)
    return in_maps


def _reference_numpy(pred, target):
    """Full numpy fallback with reference semantics (degenerate cases only)."""
    b, c, h, w = pred.shape
    n = b * h * w
    t = target.reshape(-1).astype(np.int64)
    valid = t != 255
    t0 = np.where(valid, t, 0)
    logits = np.transpose(pred, (0, 2, 3, 1)).reshape(n, c).astype(np.float32)
    m = logits.max(axis=1, keepdims=True)
    ex = np.exp(logits - m)
    s = ex.sum(axis=1)
    pt = ex[np.arange(n), t0] / s
    mask_prob = np.where(valid, pt, 1.0).astype(np.float32)
    kth = np.sort(mask_prob)[min(n, MIN_KEPT) - 1]
    thr = max(float(kth), THRESH)
    kept = mask_prob <= thr
    fv = valid & kept
    nll = (np.log(s) + m[:, 0] - logits[np.arange(n), t0]).astype(np.float32)
    num = float(np.where(fv, nll, 0.0).sum(dtype=np.float64))
    cnt = float(fv.sum())
    return np.float32(num / max(cnt, 1.0))


def _run_device(in_maps, g, trace=False):
    nc = _get_nc(g)
    return run_bass_kernel_spmd(nc, in_maps, list(range(NCORES)), trace=trace)


def kernel(pred, target):
    pred = np.asarray(pred)
    target = np.asarray(target)
    assert pred.shape == (BATCH, C, HH, WW), pred.shape
    assert target.shape == (BATCH, HH, WW), target.shape

    if target.min() < 0 or target.max() >= C:
        # ignore_index / out-of-range labels: not producible by the input
        # spec (randint 0..18); handle via the host reference for safety.
        return _reference_numpy(pred, target)

    g = GEO_FULL
    in_maps = _shard_inputs(pred, target, g)
    res = _run_device(in_maps, g).results

    num = 0.0
    cnt = 0.0
    for core in range(NCORES):
        a = res[core]["acc"].astype(np.float64)
        num += a[0, :512].sum()
        cnt += a[0, 512:].sum()
    num += NLL_MIN * cnt

    if cnt < MIN_KEPT:
        # kth-smallest prob exceeds 0.7: threshold is data-dependent.
        return _reference_numpy(pred, target)

    return np.float32(num / max(cnt, 1.0))


# revision 51
# speedup vs baseline: 1.0062x; 1.0062x over previous
"""OHEM cross-entropy loss (CriterionOhem) on 8 Trainium2 NeuronCores.

Reference semantics (N = 4*512*1024 pixels, C = 19 classes):
  p_i     = softmax(pred)[i, t_i]                (true-class prob per pixel)
  kth     = sort(p)[MIN_KEPT-1]
  thr     = max(kth, 0.7)
  keep_i  = p_i <= thr
  loss    = sum(keep_i * nll_i) / max(count(keep), 1)

Key reduction: if count(p <= 0.7) >= MIN_KEPT then kth <= 0.7 and thr == 0.7
exactly, so no top-k is needed — only a masked sum + count, which the host
verifies from the device partials (falling back to a numpy reference in the
degenerate case, which cannot occur for randn logits).

Host-side trick: per pixel, the target-class logit is swapped into class
slot 0 (a pure permutation — the softmax denominator is permutation
invariant and x_t becomes the slot-0 plane).  This removes every trace of
the target from the device program: no replicated (t - c) bytes on the
wire, no masked-logit pass on DVE, no second matmul pass on PE.

Device layout per core (262,144 pixels = 128 blocks x 2048):
  xk   [128*19, 2048] fp8    class-rows, row g = 19*block + class
  xk16 [3*128, 2048]  fp16   the three DVE_TILES' rows (DVE needs 2-byte)
  x0   [128, 2048]    fp16   slot-0 (target-class) logits + NLL_MIN
  19 tiles of 128 consecutive class-rows stream through:
    ACT: et = exp(x) for 16 tiles (chunked to amortize fixed overhead);
    DVE: et = exp(x) for 3 tiles via a bit-trick exp2 (engine balancing);
    PE : one-hot block maps contract the 19 classes of each block into
         S[block, pixel], accumulated across all 19 tiles into four
         [128, 512] fp32 PSUM banks (start@k=0 / stop@k=18).
  Finals per bank as its accumulation group stops: lnS = ln(S) on ACT;
  d1 = lnS - x0' on DVE; rl = relu(d1), kp = (d1 >= 0); PE ones-matmuls
  fold partitions into R[1, 512] accumulators across banks; host sums
  num = sum(rl) + NLL_MIN*cnt, cnt = sum(kp).
"""

import numpy as np
from contextlib import ExitStack

import concourse.bass as bass
import concourse.tile as tile
from concourse import bacc
from concourse import mybir
from concourse.bass_utils import run_bass_kernel_spmd

F32 = mybir.dt.float32
F16 = mybir.dt.float16
F8 = mybir.dt.float8e4
AF = mybir.ActivationFunctionType
OP = mybir.AluOpType

C = 19
THRESH = 0.7
MIN_KEPT = 100000
NLL_MIN = float(-np.log(np.float32(THRESH)))  # keep <=> nll >= -ln(0.7)

# Full-size geometry: 4x19x512x1024 pred over 8 cores.
BATCH, HH, WW = 4, 512, 1024
NCORES = 8


class Geo:
    def __init__(self, X, nblk):
        self.X = X                      # pixels per block (free axis)
        self.NBLK = nblk                # blocks per core (psum partitions)
        self.NROWS = C * nblk           # class-rows per core
        self.NT = self.NROWS // 128     # 128-row tiles
        assert self.NROWS % 128 == 0
        self.NPIX = nblk * X            # pixels per core


GEO_FULL = Geo(2048, 128)               # 128 x 2048 = 262,144 pixels/core


def make_consts(g):
    """Per-tile one-hot block maps, packed side by side: map k column block
    [k*128,(k+1)*128) has a one at (p, (128k+p)//19)."""
    maps = np.zeros((128, g.NT * 128), np.float16)
    for k in range(g.NT):
        for p in range(128):
            maps[p, k * 128 + (128 * k + p) // C] = 1.0
    return {"maps": maps}


ACT_TABLE_LN_EXP = 6  # natural_log_exp_and_others in act_info.json

# Tiles whose exp runs on DVE (bit-trick exp2) instead of ACT, balancing the
# two engines; spread through the stream so matmul feeding stays smooth.
DVE_TILES = (4, 9, 14)
# Minimax quadratic for 2^f on [-0.5, 0.5]: 2^f ~= S2*(f+A2)^2 + C2.
S2, A2, C2 = 0.22266791031510733, 1.556176036576733, 0.4609102972256174
L2E = float(np.log2(np.e))


def emit(ctx, tc, g, xk, xks, xk16, x0, maps, acc):
    nc = tc.nc
    X = g.X
    NSLC = X // 512
    I16 = mybir.dt.int16

    # ACT work items: chunks of tiles sharing one SBUF tile / ACT exp.
    # Singles at both ends (fast first exp; short final matmul backlog
    # behind the last exp); pairs in the middle amortize the fixed
    # per-instruction overhead.  DVE tiles are woven in between.
    act_tiles = [k for k in range(g.NT) if k not in DVE_TILES]
    chunks = [(act_tiles[0],)]
    i = 1
    while i + 2 < len(act_tiles):
        chunks.append((act_tiles[i], act_tiles[i + 1]))
        i += 2
    chunks += [(k,) for k in act_tiles[i:]]
    # DVE exp work is woven in EARLY (DVE is otherwise idle until the
    # finals), but its matmuls are deferred to the very end: PE executes
    # in order, so a DVE-gated matmul mid-stream would block all later
    # ACT-tile matmuls.  The accumulation stop therefore sits on the last
    # DVE tile.
    pair_idx = {ch: n for n, ch in
                enumerate(c for c in chunks if len(c) == 2)}
    items = []
    for ci, ch in enumerate(chunks):
        items.append(("act", ch))
        if 0 < ci <= len(DVE_TILES):
            items.append(("dve", ci - 1))

    xvp = ctx.enter_context(tc.tile_pool(name="xvp", bufs=3))
    etp = ctx.enter_context(tc.tile_pool(name="etp", bufs=3))
    cst = ctx.enter_context(tc.tile_pool(name="cst", bufs=1))
    pss = ctx.enter_context(tc.tile_pool(name="pss", bufs=1, space="PSUM"))
    fin = ctx.enter_context(tc.tile_pool(name="fin", bufs=1))
    lnp = ctx.enter_context(tc.tile_pool(name="lnp", bufs=2))
    rkp = ctx.enter_context(tc.tile_pool(name="rkp", bufs=2))
    dxp = ctx.enter_context(tc.tile_pool(name="dxp", bufs=2))
    dep = ctx.enter_context(tc.tile_pool(name="dep", bufs=len(DVE_TILES)))
    dsp = ctx.enter_context(tc.tile_pool(name="dsp", bufs=2))

    # One activation table serves both Exp and Ln; load it up front so the
    # compiler's table-load pass inserts nothing mid-stream.
    nc.scalar.add_instruction(mybir.InstLoadActFuncSet(
        name=nc.get_next_instruction_name(), ins=[], outs=[],
        act_func_set_id=ACT_TABLE_LN_EXP))

    maps_t = cst.tile([128, g.NT * 128], F16)
    x0t = cst.tile([128, X], F16)
    ones_t = cst.tile([128, 1], F16)
    nc.vector.memset(ones_t[:], 1.0)

    # One PSUM tile per 512-column bank: a PSUM reader waits for its whole
    # accumulation group, so separate tiles let each bank's finals start as
    # soon as that bank's own 19-matmul group stops.
    S_ps = [pss.tile([128, 512], F32, name=f"S{m}") for m in range(NSLC)]
    # Final partial sums: R[0,:] accumulates relu(d1) column sums, R[1,:]
    # keep-count column sums, via ones-vector matmuls over the 4 banks.
    R_rl = pss.tile([1, 512], F32)
    R_kp = pss.tile([1, 512], F32)

    def mm_tile(k, et, off, m0=0, m1=NSLC):
        lhs = maps_t[:, k * 128:(k + 1) * 128]
        for m in range(m0, m1):
            nc.tensor.matmul(S_ps[m][:, :], lhs,
                             et[:, off + m * 512:off + (m + 1) * 512],
                             start=(k == 0), stop=(k == g.NT - 1))

    def dma_item(i):
        kind, v = items[i]
        if kind == "act":
            xv = xvp.tile([128, len(v) * X], F8, tag="x")
            if len(v) == 2:
                pi = pair_idx[v]
                nc.sync.dma_start(xv[:], xk[128 * pi:128 * (pi + 1), :])
            else:
                si = 0 if v[0] == 0 else 1
                nc.sync.dma_start(xv[:], xks[128 * si:128 * (si + 1), :])
        else:
            xv = dxp.tile([128, X], F16, tag="dx")
            nc.sync.dma_start(xv[:], xk16[128 * v:128 * (v + 1), :])
        # Map loads: small per-chunk slices early (per-DMA HWDGE setup is
        # ~625ns, but a single bulk load early would displace the pixel
        # tiles that gate ACT), then one bulk DMA for tiles 8+ once the
        # queue has slack.  Each lands before the matmuls that read it.
        if i == 0:
            nc.sync.dma_start(maps_t[:, 0:128], maps[:, 0:128])
        elif i == 1:
            nc.sync.dma_start(maps_t[:, 128:384], maps[:, 128:384])
        elif i == 3:
            nc.sync.dma_start(maps_t[:, 384:768], maps[:, 384:768])
        elif i == 5:
            nc.sync.dma_start(maps_t[:, 768:1024], maps[:, 768:1024])
        elif i == 6:
            nc.sync.dma_start(maps_t[:, 1024:], maps[:, 1024:])
        if i == len(items) - 2:
            nc.sync.dma_start(x0t[:], x0)
        return xv

    def dve_exp(xv, et):
        """et = exp(xv) on DVE: 2^(x*log2 e) via the fp16 +1536 rounding
        trick for the integer part (exponent-field construction) and a
        minimax quadratic for 2^frac.  ~0.9% max rel error."""
        s1 = dsp.tile([128, X], F16, tag="s1")
        s2 = dsp.tile([128, X], F16, tag="s2")
        s3 = dsp.tile([128, X], F16, tag="s3")
        TS, TT = nc.vector.tensor_scalar, nc.vector.tensor_tensor
        TS(out=s1[:], in0=xv[:], scalar1=L2E, scalar2=1536.0,
           op0=OP.mult, op1=OP.add)                      # s1 = 1536 + n
        TS(out=s2[:], in0=s1[:], scalar1=-1536.0, scalar2=None,
           op0=OP.add)                                   # s2 = n
        TS(out=s3[:], in0=xv[:], scalar1=L2E, scalar2=None,
           op0=OP.mult)                                  # s3 = y
        TT(out=s3[:], in0=s3[:], in1=s2[:], op=OP.subtract)  # s3 = f
        TS(out=s1[:].bitcast(I16), in0=s1[:].bitcast(I16),
           scalar1=15 - 0x6600, scalar2=None, op0=OP.add)
        TS(out=s1[:].bitcast(I16), in0=s1[:].bitcast(I16),
           scalar1=10, scalar2=None, op0=OP.logical_shift_left)  # s1 = 2^n
        TS(out=s2[:], in0=s3[:], scalar1=float(A2), scalar2=None,
           op0=OP.add)                                   # s2 = f + A2
        TT(out=s2[:], in0=s2[:], in1=s2[:], op=OP.mult)  # s2 = (f+A2)^2
        TS(out=s2[:], in0=s2[:], scalar1=float(S2), scalar2=float(C2),
           op0=OP.mult, op1=OP.add)                      # s2 ~= 2^f
        TT(out=et[:], in0=s2[:], in1=s1[:], op=OP.mult)  # et = 2^f * 2^n

    # Software-pipelined: item i+1's DMAs are emitted (and queued) before
    # item i's compute so the load stream never waits on compute emission.
    dve_ets = []
    xv_next = dma_item(0)
    for i, (kind, v) in enumerate(items):
        xv = xv_next
        if i + 1 < len(items):
            xv_next = dma_item(i + 1)
        if kind == "dve":
            et = dep.tile([128, X], F16, tag="de")
            dve_exp(xv, et)
            dve_ets.append(et)
            continue
        if i == len(items) - 2:
            # Deferred DVE-tile matmuls (their ets completed mid-stream):
            # emitted here so PE digests them under the second-to-last
            # exp, leaving only the last tile's matmuls trailing.
            for m in range(NSLC):
                for di, det in enumerate(dve_ets):
                    mm_tile(DVE_TILES[di], det, 0, m, m + 1)
        W = len(v) * X
        et = etp.tile([128, W], F16, tag="e")
        # The last tile's exp runs in halves so its matmuls (which gate the
        # finals) start two PSUM banks early.
        hsplit = 2 if i == len(items) - 1 else 1
        for h in range(hsplit):
            hw = W // hsplit
            nc.scalar.activation(et[:, h * hw:(h + 1) * hw],
                                 xv[:, h * hw:(h + 1) * hw], AF.Exp)
            for j, k in enumerate(v):
                mm_tile(k, et, j * X, h * NSLC // hsplit,
                        (h + 1) * NSLC // hsplit)

    # Per-bank finals (x0 already holds x_t + NLL_MIN from the host):
    #   d1 = ln S - x0' ; rl = relu(d1) ; kp = (d1 >= 0)
    # then PE ones-matmuls fold the partition dimension into R_{rl,kp},
    # accumulating across banks; the host sums the 512-wide rows.
    for m in range(NSLC):
        sl = slice(m * 512, (m + 1) * 512)
        lnS = lnp.tile([128, 512], F16, tag="ln")
        nc.scalar.activation(lnS[:], S_ps[m][:, :], AF.Ln)
        rk = rkp.tile([128, 3, 512], F16, tag="rk")
        d1 = rk[:, 2, :]
        nc.vector.tensor_sub(d1, lnS[:], x0t[:, sl])
        nc.vector.tensor_scalar(out=rk[:, 0, :], in0=d1, scalar1=0.0,
                                scalar2=None, op0=OP.max)
        nc.vector.tensor_scalar(out=rk[:, 1, :], in0=d1, scalar1=0.0,
                                scalar2=None, op0=OP.is_ge)
        nc.tensor.matmul(R_rl[:, :], ones_t[:], rk[:, 0, :],
                         start=(m == 0), stop=(m == NSLC - 1))
        nc.tensor.matmul(R_kp[:, :], ones_t[:], rk[:, 1, :],
                         start=(m == 0), stop=(m == NSLC - 1))

    # Evacuate the two PSUM rows in parallel on ACT and DVE (both engines
    # must address partition 0 — engine partition bases are 0/32/64/96),
    # then one DMA.
    accs = fin.tile([1, 1024], F32)
    nc.scalar.copy(accs[0:1, 0:512], R_rl[:, :])
    nc.vector.tensor_scalar(out=accs[0:1, 512:1024], in0=R_kp[:, :],
                            scalar1=0.0, scalar2=None, op0=OP.add)
    nc.sync.dma_start(acc[:, :], accs[:])


def build_nc(g):
    nc = bacc.Bacc(
        "TRN2",
        target_bir_lowering=False,
        debug=False,
        enable_asserts=True,
        num_devices=NCORES,
    )
    xk = nc.dram_tensor("xk", [7 * 128, 2 * g.X], F8, kind="ExternalInput")
    xks = nc.dram_tensor("xks", [2 * 128, g.X], F8, kind="ExternalInput")
    xk16 = nc.dram_tensor("xk16", [len(DVE_TILES) * 128, g.X], F16,
                          kind="ExternalInput")
    x0 = nc.dram_tensor("x0", [g.NBLK, g.X], F16, kind="ExternalInput")
    maps = nc.dram_tensor("maps", [128, g.NT * 128], F16, kind="ExternalInput")
    acc = nc.dram_tensor("acc", [1, 1024], F32, kind="ExternalOutput")
    with tile.TileContext(nc) as tc, ExitStack() as ctx:
        emit(ctx, tc, g, xk.ap(), xks.ap(), xk16.ap(), x0.ap(), maps.ap(),
             acc.ap())
    nc.compile()
    return nc


_NC_CACHE = {}


def _get_nc(g):
    key = (g.X, g.NBLK)
    if key not in _NC_CACHE:
        _NC_CACHE[key] = build_nc(g)
    return _NC_CACHE[key]


def make_inputs(pred_slice, target_slice, g):
    """Per-core packed inputs.  xk[b*19 + c, px] holds the fp8 logits with
    the target class swapped into slot 0; x0 is the slot-0 (target-class)
    fp16 plane with the keep threshold pre-added (device computes
    d1 = ln S - x0')."""
    import ml_dtypes
    xk = np.empty((g.NBLK, C, g.X), np.float16)
    xk[:] = pred_slice.reshape(C, g.NBLK, g.X).swapaxes(0, 1)
    t = target_slice.reshape(g.NBLK, g.X).astype(np.intp)
    bi = np.arange(g.NBLK)[:, None]
    ci = np.arange(g.X)[None, :]
    xt = xk[bi, t, ci].copy()
    xk[bi, t, ci] = xk[:, 0, :]
    xk[:, 0, :] = xt
    xkr = xk.reshape(g.NROWS, g.X)
    xk8 = xkr.astype(ml_dtypes.float8_e4m3)
    act = [k for k in range(g.NT) if k not in DVE_TILES]
    pairs = []
    i = 1
    while i + 2 < len(act):
        pairs.append((act[i], act[i + 1]))
        i += 2
    xkp = np.concatenate(
        [np.concatenate([xk8[128 * a:128 * (a + 1), :],
                         xk8[128 * b:128 * (b + 1), :]], axis=1)
         for a, b in pairs])
    xksg = np.concatenate([xk8[128 * k:128 * (k + 1), :]
                           for k in (act[0], act[-1])])
    xk16 = np.concatenate([xkr[128 * k:128 * (k + 1), :] for k in DVE_TILES])
    x0 = (xt.astype(np.float32) + np.float32(NLL_MIN)).astype(np.float16)
    return {"xk": xkp, "xks": xksg, "xk16": xk16, "x0": x0}


def _shard_inputs(pred, target, g):
    """Slice the full inputs into per-core in_maps (8 cores)."""
    consts = make_consts(g)
    in_maps = []
    rows_per_core = HH // 2  # 256
    for core in range(NCORES):
        b, half = core // 2, core % 2
        h0 = half * rows_per_core
        m = make_inputs(pred[b, :, h0:h0 + rows_per_core, :],
                        target[b, h0:h0 + rows_per_core, :], g)
        m.update(consts)
        in_maps.append(m)
    return in_maps


def _reference_numpy(pred, target):
    """Full numpy fallback with reference semantics (degenerate cases only)."""
    b, c, h, w = pred.shape
    n = b * h * w
    t = target.reshape(-1).astype(np.int64)
    valid = t != 255
    t0 = np.where(valid, t, 0)
    logits = np.transpose(pred, (0, 2, 3, 1)).reshape(n, c).astype(np.float32)
    m = logits.max(axis=1, keepdims=True)
    ex = np.exp(logits - m)
    s = ex.sum(axis=1)
    pt = ex[np.arange(n), t0] / s
    mask_prob = np.where(valid, pt, 1.0).astype(np.float32)
    kth = np.sort(mask_prob)[min(n, MIN_KEPT) - 1]
    thr = max(float(kth), THRESH)
    kept = mask_prob <= thr
    fv = valid & kept
    nll = (np.log(s) + m[:, 0] - logits[np.arange(n), t0]).astype(np.float32)
    num = float(np.where(fv, nll, 0.0).sum(dtype=np.float64))
    cnt = float(fv.sum())
    return np.float32(num / max(cnt, 1.0))


def _run_device(in_maps, g, trace=False):
    nc = _get_nc(g)
    return run_bass_kernel_spmd(nc, in_maps, list(range(NCORES)), trace=trace)


def kernel(pred, target):
    pred = np.asarray(pred)
    target = np.asarray(target)
    assert pred.shape == (BATCH, C, HH, WW), pred.shape
    assert target.shape == (BATCH, HH, WW), target.shape

    if target.min() < 0 or target.max() >= C:
        # ignore_index / out-of-range labels: not producible by the input
        # spec (randint 0..18); handle via the host reference for safety.
        return _reference_numpy(pred, target)

    g = GEO_FULL
    in_maps = _shard_inputs(pred, target, g)
    res = _run_device(in_maps, g).results

    num = 0.0
    cnt = 0.0
    for core in range(NCORES):
        a = res[core]["acc"].astype(np.float64)
        num += a[0, :512].sum()
        cnt += a[0, 512:].sum()
    num += NLL_MIN * cnt

    if cnt < MIN_KEPT:
        # kth-smallest prob exceeds 0.7: threshold is data-dependent.
        return _reference_numpy(pred, target)

    return np.float32(num / max(cnt, 1.0))


# revision 54
# speedup vs baseline: 1.0133x; 1.0071x over previous
"""OHEM cross-entropy loss (CriterionOhem) on 8 Trainium2 NeuronCores.

Reference semantics (N = 4*512*1024 pixels, C = 19 classes):
  p_i     = softmax(pred)[i, t_i]                (true-class prob per pixel)
  kth     = sort(p)[MIN_KEPT-1]
  thr     = max(kth, 0.7)
  keep_i  = p_i <= thr
  loss    = sum(keep_i * nll_i) / max(count(keep), 1)

Key reduction: if count(p <= 0.7) >= MIN_KEPT then kth <= 0.7 and thr == 0.7
exactly, so no top-k is needed — only a masked sum + count, which the host
verifies from the device partials (falling back to a numpy reference in the
degenerate case, which cannot occur for randn logits).

Host-side trick: per pixel, the target-class logit is swapped into class
slot 0 (a pure permutation — the softmax denominator is permutation
invariant and x_t becomes the slot-0 plane).  This removes every trace of
the target from the device program: no replicated (t - c) bytes on the
wire, no masked-logit pass on DVE, no second matmul pass on PE.

Device layout per core (262,144 pixels = 128 blocks x 2048):
  xk   [128*19, 2048] fp8    class-rows, row g = 19*block + class
  xk16 [3*128, 2048]  fp16   the three DVE_TILES' rows (DVE needs 2-byte)
  x0   [128, 2048]    fp16   slot-0 (target-class) logits + NLL_MIN
  19 tiles of 128 consecutive class-rows stream through:
    ACT: et = exp(x) for 16 tiles (chunked to amortize fixed overhead);
    DVE: et = exp(x) for 3 tiles via a bit-trick exp2 (engine balancing);
    PE : one-hot block maps contract the 19 classes of each block into
         S[block, pixel], accumulated across all 19 tiles into four
         [128, 512] fp32 PSUM banks (start@k=0 / stop@k=18).
  Finals per bank as its accumulation group stops: lnS = ln(S) on ACT;
  d1 = lnS - x0' on DVE; rl = relu(d1), kp = (d1 >= 0); PE ones-matmuls
  fold partitions into R[1, 512] accumulators across banks; host sums
  num = sum(rl) + NLL_MIN*cnt, cnt = sum(kp).
"""

import numpy as np
from contextlib import ExitStack

import concourse.bass as bass
import concourse.tile as tile
from concourse import bacc
from concourse import mybir
from concourse.bass_utils import run_bass_kernel_spmd

F32 = mybir.dt.float32
F16 = mybir.dt.float16
F8 = mybir.dt.float8e4
AF = mybir.ActivationFunctionType
OP = mybir.AluOpType

C = 19
THRESH = 0.7
MIN_KEPT = 100000
NLL_MIN = float(-np.log(np.float32(THRESH)))  # keep <=> nll >= -ln(0.7)

# Full-size geometry: 4x19x512x1024 pred over 8 cores.
BATCH, HH, WW = 4, 512, 1024
NCORES = 8


class Geo:
    def __init__(self, X, nblk):
        self.X = X                      # pixels per block (free axis)
        self.NBLK = nblk                # blocks per core (psum partitions)
        self.NROWS = C * nblk           # class-rows per core
        self.NT = self.NROWS // 128     # 128-row tiles
        assert self.NROWS % 128 == 0
        self.NPIX = nblk * X            # pixels per core


GEO_FULL = Geo(2048, 128)               # 128 x 2048 = 262,144 pixels/core


def make_consts(g):
    """Per-tile one-hot block maps, packed side by side: map k column block
    [k*128,(k+1)*128) has a one at (p, (128k+p)//19)."""
    maps = np.zeros((128, g.NT * 128), np.float16)
    for k in range(g.NT):
        for p in range(128):
            maps[p, k * 128 + (128 * k + p) // C] = 1.0
    return {"maps": maps}


ACT_TABLE_LN_EXP = 6  # natural_log_exp_and_others in act_info.json

# Tiles whose exp runs on DVE (bit-trick exp2) instead of ACT, balancing the
# two engines; spread through the stream so matmul feeding stays smooth.
DVE_TILES = (4, 9, 14)
# Minimax quadratic for 2^f on [-0.5, 0.5]: 2^f ~= S2*(f+A2)^2 + C2.
S2, A2, C2 = 0.22266791031510733, 1.556176036576733, 0.4609102972256174
L2E = float(np.log2(np.e))


def _chunk_plan(g):
    """Shared ACT chunking rule: leading single, pairs, three trailing
    singles (their matmuls hide under the following exps)."""
    act = [k for k in range(g.NT) if k not in DVE_TILES]
    pairs = []
    i = 1
    while i + 3 < len(act):
        pairs.append((act[i], act[i + 1]))
        i += 2
    singles = [act[0]] + act[i:]
    return pairs, singles


def emit(ctx, tc, g, xk, xks, xk16, x0, maps, acc):
    nc = tc.nc
    X = g.X
    NSLC = X // 512
    I16 = mybir.dt.int16

    # ACT work items: chunks of tiles sharing one SBUF tile / ACT exp.
    # Singles at both ends (fast first exp; short final matmul backlog
    # behind the last exp); pairs in the middle amortize the fixed
    # per-instruction overhead.  DVE tiles are woven in between.
    act_tiles = [k for k in range(g.NT) if k not in DVE_TILES]
    chunks = [(act_tiles[0],)]
    i = 1
    while i + 3 < len(act_tiles):
        chunks.append((act_tiles[i], act_tiles[i + 1]))
        i += 2
    chunks += [(k,) for k in act_tiles[i:]]
    # DVE exp work is woven in EARLY (DVE is otherwise idle until the
    # finals), but its matmuls are deferred to the very end: PE executes
    # in order, so a DVE-gated matmul mid-stream would block all later
    # ACT-tile matmuls.  The accumulation stop therefore sits on the last
    # DVE tile.
    pair_idx = {ch: n for n, ch in
                enumerate(c for c in chunks if len(c) == 2)}
    single_idx = {c[0]: n for n, c in
                  enumerate(c for c in chunks if len(c) == 1)}
    items = []
    for ci, ch in enumerate(chunks):
        items.append(("act", ch))
        if 0 < ci <= len(DVE_TILES):
            items.append(("dve", ci - 1))

    xvp = ctx.enter_context(tc.tile_pool(name="xvp", bufs=3))
    etp = ctx.enter_context(tc.tile_pool(name="etp", bufs=3))
    cst = ctx.enter_context(tc.tile_pool(name="cst", bufs=1))
    pss = ctx.enter_context(tc.tile_pool(name="pss", bufs=1, space="PSUM"))
    fin = ctx.enter_context(tc.tile_pool(name="fin", bufs=1))
    lnp = ctx.enter_context(tc.tile_pool(name="lnp", bufs=2))
    rkp = ctx.enter_context(tc.tile_pool(name="rkp", bufs=2))
    dxp = ctx.enter_context(tc.tile_pool(name="dxp", bufs=2))
    dep = ctx.enter_context(tc.tile_pool(name="dep", bufs=len(DVE_TILES)))
    dsp = ctx.enter_context(tc.tile_pool(name="dsp", bufs=2))

    # One activation table serves both Exp and Ln; load it up front so the
    # compiler's table-load pass inserts nothing mid-stream.
    nc.scalar.add_instruction(mybir.InstLoadActFuncSet(
        name=nc.get_next_instruction_name(), ins=[], outs=[],
        act_func_set_id=ACT_TABLE_LN_EXP))

    maps_t = cst.tile([128, g.NT * 128], F16)
    x0t = cst.tile([128, X], F16)
    ones_t = cst.tile([128, 1], F16)
    nc.vector.memset(ones_t[:], 1.0)

    # One PSUM tile per 512-column bank: a PSUM reader waits for its whole
    # accumulation group, so separate tiles let each bank's finals start as
    # soon as that bank's own 19-matmul group stops.
    S_ps = [pss.tile([128, 512], F32, name=f"S{m}") for m in range(NSLC)]
    # Final partial sums: R[0,:] accumulates relu(d1) column sums, R[1,:]
    # keep-count column sums, via ones-vector matmuls over the 4 banks.
    R_rl = pss.tile([1, 512], F32)
    R_kp = pss.tile([1, 512], F32)

    def mm_tile(k, et, off, m0=0, m1=NSLC):
        lhs = maps_t[:, k * 128:(k + 1) * 128]
        for m in range(m0, m1):
            nc.tensor.matmul(S_ps[m][:, :], lhs,
                             et[:, off + m * 512:off + (m + 1) * 512],
                             start=(k == 0), stop=(k == g.NT - 1))

    def dma_item(i):
        kind, v = items[i]
        if kind == "act":
            xv = xvp.tile([128, len(v) * X], F8, tag="x")
            if len(v) == 2:
                pi = pair_idx[v]
                nc.sync.dma_start(xv[:], xk[128 * pi:128 * (pi + 1), :])
            else:
                si = single_idx[v[0]]
                nc.sync.dma_start(xv[:], xks[128 * si:128 * (si + 1), :])
        else:
            xv = dxp.tile([128, X], F16, tag="dx")
            nc.sync.dma_start(xv[:], xk16[128 * v:128 * (v + 1), :])
        # Map loads: small per-chunk slices early (per-DMA HWDGE setup is
        # ~625ns, but a single bulk load early would displace the pixel
        # tiles that gate ACT), then one bulk DMA for tiles 8+ once the
        # queue has slack.  Each lands before the matmuls that read it.
        if i == 0:
            nc.sync.dma_start(maps_t[:, 0:128], maps[:, 0:128])
        elif i == 1:
            nc.sync.dma_start(maps_t[:, 128:384], maps[:, 128:384])
        elif i == 3:
            nc.sync.dma_start(maps_t[:, 384:768], maps[:, 384:768])
        elif i == 5:
            nc.sync.dma_start(maps_t[:, 768:1024], maps[:, 768:1024])
        elif i == 6:
            nc.sync.dma_start(maps_t[:, 1024:], maps[:, 1024:])
        if i == len(items) - 2:
            nc.sync.dma_start(x0t[:], x0)
        return xv

    def dve_exp(xv, et):
        """et = exp(xv) on DVE: 2^(x*log2 e) via the fp16 +1536 rounding
        trick for the integer part (exponent-field construction) and a
        minimax quadratic for 2^frac.  ~0.9% max rel error."""
        s1 = dsp.tile([128, X], F16, tag="s1")
        s2 = dsp.tile([128, X], F16, tag="s2")
        s3 = dsp.tile([128, X], F16, tag="s3")
        TS, TT = nc.vector.tensor_scalar, nc.vector.tensor_tensor
        TS(out=s1[:], in0=xv[:], scalar1=L2E, scalar2=1536.0,
           op0=OP.mult, op1=OP.add)                      # s1 = 1536 + n
        TS(out=s2[:], in0=s1[:], scalar1=-1536.0, scalar2=None,
           op0=OP.add)                                   # s2 = n
        TS(out=s3[:], in0=xv[:], scalar1=L2E, scalar2=None,
           op0=OP.mult)                                  # s3 = y
        TT(out=s3[:], in0=s3[:], in1=s2[:], op=OP.subtract)  # s3 = f
        TS(out=s1[:].bitcast(I16), in0=s1[:].bitcast(I16),
           scalar1=15 - 0x6600, scalar2=None, op0=OP.add)
        TS(out=s1[:].bitcast(I16), in0=s1[:].bitcast(I16),
           scalar1=10, scalar2=None, op0=OP.logical_shift_left)  # s1 = 2^n
        TS(out=s2[:], in0=s3[:], scalar1=float(A2), scalar2=None,
           op0=OP.add)                                   # s2 = f + A2
        TT(out=s2[:], in0=s2[:], in1=s2[:], op=OP.mult)  # s2 = (f+A2)^2
        TS(out=s2[:], in0=s2[:], scalar1=float(S2), scalar2=float(C2),
           op0=OP.mult, op1=OP.add)                      # s2 ~= 2^f
        TT(out=et[:], in0=s2[:], in1=s1[:], op=OP.mult)  # et = 2^f * 2^n

    # Software-pipelined: item i+1's DMAs are emitted (and queued) before
    # item i's compute so the load stream never waits on compute emission.
    dve_ets = []
    xv_next = dma_item(0)
    for i, (kind, v) in enumerate(items):
        xv = xv_next
        if i + 1 < len(items):
            xv_next = dma_item(i + 1)
        if kind == "dve":
            et = dep.tile([128, X], F16, tag="de")
            dve_exp(xv, et)
            dve_ets.append(et)
            continue
        if i == len(items) - 2:
            # Deferred DVE-tile matmuls (their ets completed mid-stream):
            # emitted here so PE digests them under the second-to-last
            # exp, leaving only the last tile's matmuls trailing.
            for m in range(NSLC):
                for di, det in enumerate(dve_ets):
                    mm_tile(DVE_TILES[di], det, 0, m, m + 1)
        W = len(v) * X
        et = etp.tile([128, W], F16, tag="e")
        # The last tile's exp runs in halves so its matmuls (which gate the
        # finals) start two PSUM banks early.
        hsplit = 2 if i == len(items) - 1 else 1
        for h in range(hsplit):
            hw = W // hsplit
            nc.scalar.activation(et[:, h * hw:(h + 1) * hw],
                                 xv[:, h * hw:(h + 1) * hw], AF.Exp)
            for j, k in enumerate(v):
                mm_tile(k, et, j * X, h * NSLC // hsplit,
                        (h + 1) * NSLC // hsplit)

    # Per-bank finals (x0 already holds x_t + NLL_MIN from the host):
    #   d1 = ln S - x0' ; rl = relu(d1) ; kp = (d1 >= 0)
    # then PE ones-matmuls fold the partition dimension into R_{rl,kp},
    # accumulating across banks; the host sums the 512-wide rows.
    for m in range(NSLC):
        sl = slice(m * 512, (m + 1) * 512)
        lnS = lnp.tile([128, 512], F16, tag="ln")
        nc.scalar.activation(lnS[:], S_ps[m][:, :], AF.Ln)
        rk = rkp.tile([128, 3, 512], F16, tag="rk")
        d1 = rk[:, 2, :]
        nc.vector.tensor_sub(d1, lnS[:], x0t[:, sl])
        nc.vector.tensor_scalar(out=rk[:, 0, :], in0=d1, scalar1=0.0,
                                scalar2=None, op0=OP.max)
        nc.vector.tensor_scalar(out=rk[:, 1, :], in0=d1, scalar1=0.0,
                                scalar2=None, op0=OP.is_ge)
        nc.tensor.matmul(R_rl[:, :], ones_t[:], rk[:, 0, :],
                         start=(m == 0), stop=(m == NSLC - 1))
        nc.tensor.matmul(R_kp[:, :], ones_t[:], rk[:, 1, :],
                         start=(m == 0), stop=(m == NSLC - 1))

    # Evacuate the two PSUM rows in parallel on ACT and DVE (both engines
    # must address partition 0 — engine partition bases are 0/32/64/96),
    # then one DMA.
    accs = fin.tile([1, 1024], F32)
    nc.scalar.copy(accs[0:1, 0:512], R_rl[:, :])
    nc.vector.tensor_scalar(out=accs[0:1, 512:1024], in0=R_kp[:, :],
                            scalar1=0.0, scalar2=None, op0=OP.add)
    nc.sync.dma_start(acc[:, :], accs[:])


def build_nc(g):
    nc = bacc.Bacc(
        "TRN2",
        target_bir_lowering=False,
        debug=False,
        enable_asserts=True,
        num_devices=NCORES,
    )
    pairs, singles = _chunk_plan(g)
    xk = nc.dram_tensor("xk", [len(pairs) * 128, 2 * g.X], F8,
                        kind="ExternalInput")
    xks = nc.dram_tensor("xks", [len(singles) * 128, g.X], F8,
                         kind="ExternalInput")
    xk16 = nc.dram_tensor("xk16", [len(DVE_TILES) * 128, g.X], F16,
                          kind="ExternalInput")
    x0 = nc.dram_tensor("x0", [g.NBLK, g.X], F16, kind="ExternalInput")
    maps = nc.dram_tensor("maps", [128, g.NT * 128], F16, kind="ExternalInput")
    acc = nc.dram_tensor("acc", [1, 1024], F32, kind="ExternalOutput")
    with tile.TileContext(nc) as tc, ExitStack() as ctx:
        emit(ctx, tc, g, xk.ap(), xks.ap(), xk16.ap(), x0.ap(), maps.ap(),
             acc.ap())
    nc.compile()
    return nc


_NC_CACHE = {}


def _get_nc(g):
    key = (g.X, g.NBLK)
    if key not in _NC_CACHE:
        _NC_CACHE[key] = build_nc(g)
    return _NC_CACHE[key]


def make_inputs(pred_slice, target_slice, g):
    """Per-core packed inputs.  xk[b*19 + c, px] holds the fp8 logits with
    the target class swapped into slot 0; x0 is the slot-0 (target-class)
    fp16 plane with the keep threshold pre-added (device computes
    d1 = ln S - x0')."""
    import ml_dtypes
    xk = np.empty((g.NBLK, C, g.X), np.float16)
    xk[:] = pred_slice.reshape(C, g.NBLK, g.X).swapaxes(0, 1)
    t = target_slice.reshape(g.NBLK, g.X).astype(np.intp)
    bi = np.arange(g.NBLK)[:, None]
    ci = np.arange(g.X)[None, :]
    xt = xk[bi, t, ci].copy()
    xk[bi, t, ci] = xk[:, 0, :]
    xk[:, 0, :] = xt
    xkr = xk.reshape(g.NROWS, g.X)
    xk8 = xkr.astype(ml_dtypes.float8_e4m3)
    pairs, singles = _chunk_plan(g)
    xkp = np.concatenate(
        [np.concatenate([xk8[128 * a:128 * (a + 1), :],
                         xk8[128 * b:128 * (b + 1), :]], axis=1)
         for a, b in pairs])
    xksg = np.concatenate([xk8[128 * k:128 * (k + 1), :]
                           for k in singles])
    xk16 = np.concatenate([xkr[128 * k:128 * (k + 1), :] for k in DVE_TILES])
    x0 = (xt.astype(np.float32) + np.float32(NLL_MIN)).astype(np.float16)
    return {"xk": xkp, "xks": xksg, "xk16": xk16, "x0": x0}


def _shard_inputs(pred, target, g):
    """Slice the full inputs into per-core in_maps (8 cores)."""
    consts = make_consts(g)
    in_maps = []
    rows_per_core = HH // 2  # 256
    for core in range(NCORES):
        b, half = core // 2, core % 2
        h0 = half * rows_per_core
        m = make_inputs(pred[b, :, h0:h0 + rows_per_core, :],
                        target[b, h0:h0 + rows_per_core, :], g)
        m.update(consts)
        in_maps.append(m)
    return in_maps


def _reference_numpy(pred, target):
    """Full numpy fallback with reference semantics (degenerate cases only)."""
    b, c, h, w = pred.shape
    n = b * h * w
    t = target.reshape(-1).astype(np.int64)
    valid = t != 255
    t0 = np.where(valid, t, 0)
    logits = np.transpose(pred, (0, 2, 3, 1)).reshape(n, c).astype(np.float32)
    m = logits.max(axis=1, keepdims=True)
    ex = np.exp(logits - m)
    s = ex.sum(axis=1)
    pt = ex[np.arange(n), t0] / s
    mask_prob = np.where(valid, pt, 1.0).astype(np.float32)
    kth = np.sort(mask_prob)[min(n, MIN_KEPT) - 1]
    thr = max(float(kth), THRESH)
    kept = mask_prob <= thr
    fv = valid & kept
    nll = (np.log(s) + m[:, 0] - logits[np.arange(n), t0]).astype(np.float32)
    num = float(np.where(fv, nll, 0.0).sum(dtype=np.float64))
    cnt = float(fv.sum())
    return np.float32(num / max(cnt, 1.0))


def _run_device(in_maps, g, trace=False):
    nc = _get_nc(g)
    return run_bass_kernel_spmd(nc, in_maps, list(range(NCORES)), trace=trace)


def kernel(pred, target):
    pred = np.asarray(pred)
    target = np.asarray(target)
    assert pred.shape == (BATCH, C, HH, WW), pred.shape
    assert target.shape == (BATCH, HH, WW), target.shape

    if target.min() < 0 or target.max() >= C:
        # ignore_index / out-of-range labels: not producible by the input
        # spec (randint 0..18); handle via the host reference for safety.
        return _reference_numpy(pred, target)

    g = GEO_FULL
    in_maps = _shard_inputs(pred, target, g)
    res = _run_device(in_maps, g).results

    num = 0.0
    cnt = 0.0
    for core in range(NCORES):
        a = res[core]["acc"].astype(np.float64)
        num += a[0, :512].sum()
        cnt += a[0, 512:].sum()
    num += NLL_MIN * cnt

    if cnt < MIN_KEPT:
        # kth-smallest prob exceeds 0.7: threshold is data-dependent.
        return _reference_numpy(pred, target)

    return np.float32(num / max(cnt, 1.0))


# revision 58
# speedup vs baseline: 1.0158x; 1.0025x over previous
"""OHEM cross-entropy loss (CriterionOhem) on 8 Trainium2 NeuronCores.

Reference semantics (N = 4*512*1024 pixels, C = 19 classes):
  p_i     = softmax(pred)[i, t_i]                (true-class prob per pixel)
  kth     = sort(p)[MIN_KEPT-1]
  thr     = max(kth, 0.7)
  keep_i  = p_i <= thr
  loss    = sum(keep_i * nll_i) / max(count(keep), 1)

Key reduction: if count(p <= 0.7) >= MIN_KEPT then kth <= 0.7 and thr == 0.7
exactly, so no top-k is needed — only a masked sum + count, which the host
verifies from the device partials (falling back to a numpy reference in the
degenerate case, which cannot occur for randn logits).

Host-side trick: per pixel, the target-class logit is swapped into class
slot 0 (a pure permutation — the softmax denominator is permutation
invariant and x_t becomes the slot-0 plane).  This removes every trace of
the target from the device program: no replicated (t - c) bytes on the
wire, no masked-logit pass on DVE, no second matmul pass on PE.

Device layout per core (262,144 pixels = 128 blocks x 2048):
  xk   [128*19, 2048] fp8    class-rows, row g = 19*block + class
  xk16 [3*128, 2048]  fp16   the three DVE_TILES' rows (DVE needs 2-byte)
  x0   [128, 2048]    fp16   slot-0 (target-class) logits + NLL_MIN
  19 tiles of 128 consecutive class-rows stream through:
    ACT: et = exp(x) for 16 tiles (chunked to amortize fixed overhead);
    DVE: et = exp(x) for 3 tiles via a bit-trick exp2 (engine balancing);
    PE : one-hot block maps contract the 19 classes of each block into
         S[block, pixel], accumulated across all 19 tiles into four
         [128, 512] fp32 PSUM banks (start@k=0 / stop@k=18).
  Finals per bank as its accumulation group stops: lnS = ln(S) on ACT;
  d1 = lnS - x0' on DVE; rl = relu(d1), kp = (d1 >= 0); PE ones-matmuls
  fold partitions into R[1, 512] accumulators across banks; host sums
  num = sum(rl) + NLL_MIN*cnt, cnt = sum(kp).
"""

import numpy as np
from contextlib import ExitStack

import concourse.bass as bass
import concourse.tile as tile
from concourse import bacc
from concourse import mybir
from concourse.bass_utils import run_bass_kernel_spmd

F32 = mybir.dt.float32
F16 = mybir.dt.float16
F8 = mybir.dt.float8e4
AF = mybir.ActivationFunctionType
OP = mybir.AluOpType

C = 19
THRESH = 0.7
MIN_KEPT = 100000
NLL_MIN = float(-np.log(np.float32(THRESH)))  # keep <=> nll >= -ln(0.7)

# Full-size geometry: 4x19x512x1024 pred over 8 cores.
BATCH, HH, WW = 4, 512, 1024
NCORES = 8


class Geo:
    def __init__(self, X, nblk):
        self.X = X                      # pixels per block (free axis)
        self.NBLK = nblk                # blocks per core (psum partitions)
        self.NROWS = C * nblk           # class-rows per core
        self.NT = self.NROWS // 128     # 128-row tiles
        assert self.NROWS % 128 == 0
        self.NPIX = nblk * X            # pixels per core


GEO_FULL = Geo(2048, 128)               # 128 x 2048 = 262,144 pixels/core


def make_consts(g):
    """Per-tile one-hot block maps, packed side by side: map k column block
    [k*128,(k+1)*128) has a one at (p, (128k+p)//19)."""
    maps = np.zeros((128, g.NT * 128), np.float16)
    for k in range(g.NT):
        for p in range(128):
            maps[p, k * 128 + (128 * k + p) // C] = 1.0
    return {"maps": maps}


ACT_TABLE_LN_EXP = 6  # natural_log_exp_and_others in act_info.json

# Tiles whose exp runs on DVE (bit-trick exp2) instead of ACT, balancing the
# two engines; spread through the stream so matmul feeding stays smooth.
DVE_TILES = (4, 9, 14)
# Minimax quadratic for 2^f on [-0.5, 0.5]: 2^f ~= S2*(f+A2)^2 + C2.
S2, A2, C2 = 0.22266791031510733, 1.556176036576733, 0.4609102972256174
L2E = float(np.log2(np.e))


def _chunk_plan(g):
    """Shared ACT chunking rule: leading single, pairs, three trailing
    singles (their matmuls hide under the following exps)."""
    act = [k for k in range(g.NT) if k not in DVE_TILES]
    pairs = []
    i = 1
    while i + 3 < len(act):
        pairs.append((act[i], act[i + 1]))
        i += 2
    singles = [act[0]] + act[i:]
    return pairs, singles


def emit(ctx, tc, g, xk, xks, xk16, x0, maps, acc):
    nc = tc.nc
    X = g.X
    NSLC = X // 512
    I16 = mybir.dt.int16

    # ACT work items: chunks of tiles sharing one SBUF tile / ACT exp.
    # A leading single (fast first exp) and three trailing singles (each
    # tile's matmuls hide under the next tile's exp, shrinking the PE
    # drain that gates the finals); pairs in the middle amortize the
    # fixed per-instruction overhead.  DVE tiles are woven in between.
    act_tiles = [k for k in range(g.NT) if k not in DVE_TILES]
    chunks = [(act_tiles[0],)]
    i = 1
    while i + 3 < len(act_tiles):
        chunks.append((act_tiles[i], act_tiles[i + 1]))
        i += 2
    chunks += [(k,) for k in act_tiles[i:]]
    # DVE exp work is woven in early (DVE is otherwise idle until the
    # finals); its matmuls are deferred to the second-to-last item so PE
    # digests them under the trailing exps (PE executes in order, so a
    # DVE-gated matmul mid-stream would block later ACT-tile matmuls).
    pair_idx = {ch: n for n, ch in
                enumerate(c for c in chunks if len(c) == 2)}
    single_idx = {c[0]: n for n, c in
                  enumerate(c for c in chunks if len(c) == 1)}
    items = []
    for ci, ch in enumerate(chunks):
        items.append(("act", ch))
        if 0 < ci <= len(DVE_TILES):
            items.append(("dve", ci - 1))

    xvp = ctx.enter_context(tc.tile_pool(name="xvp", bufs=4))
    etp = ctx.enter_context(tc.tile_pool(name="etp", bufs=4))
    cst = ctx.enter_context(tc.tile_pool(name="cst", bufs=1))
    pss = ctx.enter_context(tc.tile_pool(name="pss", bufs=1, space="PSUM"))
    fin = ctx.enter_context(tc.tile_pool(name="fin", bufs=1))
    lnp = ctx.enter_context(tc.tile_pool(name="lnp", bufs=4))
    rkp = ctx.enter_context(tc.tile_pool(name="rkp", bufs=4))
    dxp = ctx.enter_context(tc.tile_pool(name="dxp", bufs=2))
    dep = ctx.enter_context(tc.tile_pool(name="dep", bufs=len(DVE_TILES)))
    dsp = ctx.enter_context(tc.tile_pool(name="dsp", bufs=2))

    # One activation table serves both Exp and Ln; load it up front so the
    # compiler's table-load pass inserts nothing mid-stream.
    nc.scalar.add_instruction(mybir.InstLoadActFuncSet(
        name=nc.get_next_instruction_name(), ins=[], outs=[],
        act_func_set_id=ACT_TABLE_LN_EXP))

    maps_t = cst.tile([128, g.NT * 128], F16)
    x0t = cst.tile([128, X], F16)
    ones_t = cst.tile([128, 1], F16)
    nc.vector.memset(ones_t[:], 1.0)

    # One PSUM tile per 512-column bank: a PSUM reader waits for its whole
    # accumulation group, so separate tiles let each bank's finals start as
    # soon as that bank's own 19-matmul group stops.
    S_ps = [pss.tile([128, 512], F32, name=f"S{m}") for m in range(NSLC)]
    # Final partial sums: R[0,:] accumulates relu(d1) column sums, R[1,:]
    # keep-count column sums, via ones-vector matmuls over the 4 banks.
    R_rl = pss.tile([1, 512], F32)
    R_kp = pss.tile([1, 512], F32)

    def mm_tile(k, et, off, m0=0, m1=NSLC):
        lhs = maps_t[:, k * 128:(k + 1) * 128]
        for m in range(m0, m1):
            nc.tensor.matmul(S_ps[m][:, :], lhs,
                             et[:, off + m * 512:off + (m + 1) * 512],
                             start=(k == 0), stop=(k == g.NT - 1))

    def dma_item(i):
        kind, v = items[i]
        if kind == "act":
            xv = xvp.tile([128, len(v) * X], F8, tag="x")
            if len(v) == 2:
                pi = pair_idx[v]
                nc.sync.dma_start(xv[:], xk[128 * pi:128 * (pi + 1), :])
            else:
                si = single_idx[v[0]]
                nc.sync.dma_start(xv[:], xks[128 * si:128 * (si + 1), :])
        else:
            xv = dxp.tile([128, X], F16, tag="dx")
            nc.sync.dma_start(xv[:], xk16[128 * v:128 * (v + 1), :])
        # Map loads: small per-chunk slices early (per-DMA HWDGE setup is
        # ~625ns, but a single bulk load early would displace the pixel
        # tiles that gate ACT), then one bulk DMA for tiles 8+ once the
        # queue has slack.  Each lands before the matmuls that read it.
        if i == 0:
            nc.sync.dma_start(maps_t[:, 0:128], maps[:, 0:128])
        elif i == 1:
            nc.sync.dma_start(maps_t[:, 128:384], maps[:, 128:384])
        elif i == 3:
            nc.sync.dma_start(maps_t[:, 384:768], maps[:, 384:768])
        elif i == 5:
            nc.sync.dma_start(maps_t[:, 768:1024], maps[:, 768:1024])
        elif i == 6:
            nc.sync.dma_start(maps_t[:, 1024:], maps[:, 1024:])
        if i == len(items) - 2:
            nc.sync.dma_start(x0t[:], x0)
        return xv

    def dve_exp(xv, et):
        """et = exp(xv) on DVE: 2^(x*log2 e) via the fp16 +1536 rounding
        trick for the integer part (exponent-field construction) and a
        minimax quadratic for 2^frac.  ~0.9% max rel error."""
        s1 = dsp.tile([128, X], F16, tag="s1")
        s2 = dsp.tile([128, X], F16, tag="s2")
        s3 = dsp.tile([128, X], F16, tag="s3")
        TS, TT = nc.vector.tensor_scalar, nc.vector.tensor_tensor
        TS(out=s1[:], in0=xv[:], scalar1=L2E, scalar2=1536.0,
           op0=OP.mult, op1=OP.add)                      # s1 = 1536 + n
        TS(out=s2[:], in0=s1[:], scalar1=-1536.0, scalar2=None,
           op0=OP.add)                                   # s2 = n
        TS(out=s3[:], in0=xv[:], scalar1=L2E, scalar2=None,
           op0=OP.mult)                                  # s3 = y
        TT(out=s3[:], in0=s3[:], in1=s2[:], op=OP.subtract)  # s3 = f
        TS(out=s1[:].bitcast(I16), in0=s1[:].bitcast(I16),
           scalar1=15 - 0x6600, scalar2=None, op0=OP.add)
        TS(out=s1[:].bitcast(I16), in0=s1[:].bitcast(I16),
           scalar1=10, scalar2=None, op0=OP.logical_shift_left)  # s1 = 2^n
        TS(out=s2[:], in0=s3[:], scalar1=float(A2), scalar2=None,
           op0=OP.add)                                   # s2 = f + A2
        TT(out=s2[:], in0=s2[:], in1=s2[:], op=OP.mult)  # s2 = (f+A2)^2
        TS(out=s2[:], in0=s2[:], scalar1=float(S2), scalar2=float(C2),
           op0=OP.mult, op1=OP.add)                      # s2 ~= 2^f
        TT(out=et[:], in0=s2[:], in1=s1[:], op=OP.mult)  # et = 2^f * 2^n

    # Software-pipelined: item i+1's DMAs are emitted (and queued) before
    # item i's compute so the load stream never waits on compute emission.
    dve_ets = []
    xv_next = dma_item(0)
    for i, (kind, v) in enumerate(items):
        xv = xv_next
        if i + 1 < len(items):
            xv_next = dma_item(i + 1)
        if kind == "dve":
            et = dep.tile([128, X], F16, tag="de")
            dve_exp(xv, et)
            dve_ets.append(et)
            continue
        if i == len(items) - 2:
            # Deferred DVE-tile matmuls (their ets completed mid-stream):
            # emitted here so PE digests them under the second-to-last
            # exp, leaving only the last tile's matmuls trailing.
            for m in range(NSLC):
                for di, det in enumerate(dve_ets):
                    mm_tile(DVE_TILES[di], det, 0, m, m + 1)
        W = len(v) * X
        et = etp.tile([128, W], F16, tag="e")
        # The last tile's exp runs in halves so its matmuls (which gate the
        # finals) start two PSUM banks early.
        hsplit = 2 if i == len(items) - 1 else 1
        for h in range(hsplit):
            hw = W // hsplit
            nc.scalar.activation(et[:, h * hw:(h + 1) * hw],
                                 xv[:, h * hw:(h + 1) * hw], AF.Exp)
            for j, k in enumerate(v):
                mm_tile(k, et, j * X, h * NSLC // hsplit,
                        (h + 1) * NSLC // hsplit)

    # Per-bank finals (x0 already holds x_t + NLL_MIN from the host):
    #   d1 = ln S - x0' ; rl = relu(d1) ; kp = (d1 >= 0)
    # then PE ones-matmuls fold the partition dimension into R_{rl,kp},
    # accumulating across banks; the host sums the 512-wide rows.
    for m in range(NSLC):
        sl = slice(m * 512, (m + 1) * 512)
        lnS = lnp.tile([128, 512], F16, tag="ln")
        nc.scalar.activation(lnS[:], S_ps[m][:, :], AF.Ln)
        rk = rkp.tile([128, 3, 512], F16, tag="rk")
        d1 = rk[:, 2, :]
        nc.vector.tensor_sub(d1, lnS[:], x0t[:, sl])
        nc.vector.tensor_scalar(out=rk[:, 0, :], in0=d1, scalar1=0.0,
                                scalar2=None, op0=OP.max)
        nc.vector.tensor_scalar(out=rk[:, 1, :], in0=d1, scalar1=0.0,
                                scalar2=None, op0=OP.is_ge)
        nc.tensor.matmul(R_rl[:, :], ones_t[:], rk[:, 0, :],
                         start=(m == 0), stop=(m == NSLC - 1))
        nc.tensor.matmul(R_kp[:, :], ones_t[:], rk[:, 1, :],
                         start=(m == 0), stop=(m == NSLC - 1))

    # Evacuate the two PSUM rows in parallel on ACT and DVE (both engines
    # must address partition 0 — engine partition bases are 0/32/64/96),
    # then one DMA.
    accs = fin.tile([1, 1024], F32)
    nc.scalar.copy(accs[0:1, 0:512], R_rl[:, :])
    nc.vector.tensor_scalar(out=accs[0:1, 512:1024], in0=R_kp[:, :],
                            scalar1=0.0, scalar2=None, op0=OP.add)
    nc.sync.dma_start(acc[:, :], accs[:])


def build_nc(g):
    nc = bacc.Bacc(
        "TRN2",
        target_bir_lowering=False,
        debug=False,
        enable_asserts=True,
        num_devices=NCORES,
    )
    pairs, singles = _chunk_plan(g)
    xk = nc.dram_tensor("xk", [len(pairs) * 128, 2 * g.X], F8,
                        kind="ExternalInput")
    xks = nc.dram_tensor("xks", [len(singles) * 128, g.X], F8,
                         kind="ExternalInput")
    xk16 = nc.dram_tensor("xk16", [len(DVE_TILES) * 128, g.X], F16,
                          kind="ExternalInput")
    x0 = nc.dram_tensor("x0", [g.NBLK, g.X], F16, kind="ExternalInput")
    maps = nc.dram_tensor("maps", [128, g.NT * 128], F16, kind="ExternalInput")
    acc = nc.dram_tensor("acc", [1, 1024], F32, kind="ExternalOutput")
    with tile.TileContext(nc) as tc, ExitStack() as ctx:
        emit(ctx, tc, g, xk.ap(), xks.ap(), xk16.ap(), x0.ap(), maps.ap(),
             acc.ap())
    nc.compile()
    return nc


_NC_CACHE = {}


def _get_nc(g):
    key = (g.X, g.NBLK)
    if key not in _NC_CACHE:
        _NC_CACHE[key] = build_nc(g)
    return _NC_CACHE[key]


def make_inputs(pred_slice, target_slice, g):
    """Per-core packed inputs.  xk[b*19 + c, px] holds the fp8 logits with
    the target class swapped into slot 0; x0 is the slot-0 (target-class)
    fp16 plane with the keep threshold pre-added (device computes
    d1 = ln S - x0')."""
    import ml_dtypes
    xk = np.empty((g.NBLK, C, g.X), np.float16)
    xk[:] = pred_slice.reshape(C, g.NBLK, g.X).swapaxes(0, 1)
    t = target_slice.reshape(g.NBLK, g.X).astype(np.intp)
    bi = np.arange(g.NBLK)[:, None]
    ci = np.arange(g.X)[None, :]
    xt = xk[bi, t, ci].copy()
    xk[bi, t, ci] = xk[:, 0, :]
    xk[:, 0, :] = xt
    xkr = xk.reshape(g.NROWS, g.X)
    xk8 = xkr.astype(ml_dtypes.float8_e4m3)
    pairs, singles = _chunk_plan(g)
    xkp = np.concatenate(
        [np.concatenate([xk8[128 * a:128 * (a + 1), :],
                         xk8[128 * b:128 * (b + 1), :]], axis=1)
         for a, b in pairs])
    xksg = np.concatenate([xk8[128 * k:128 * (k + 1), :]
                           for k in singles])
    xk16 = np.concatenate([xkr[128 * k:128 * (k + 1), :] for k in DVE_TILES])
    x0 = (xt.astype(np.float32) + np.float32(NLL_MIN)).astype(np.float16)
    return {"xk": xkp, "xks": xksg, "xk16": xk16, "x0": x0}


def _shard_inputs(pred, target, g):
    """Slice the full inputs into per-core in_maps (8 cores)."""
    consts = make_consts(g)
    in_maps = []
    rows_per_core = HH // 2  # 256
    for core in range(NCORES):
        b, half = core // 2, core % 2
        h0 = half * rows_per_core
        m = make_inputs(pred[b, :, h0:h0 + rows_per_core, :],
                        target[b, h0:h0 + rows_per_core, :], g)
        m.update(consts)
        in_maps.append(m)
    return in_maps


def _reference_numpy(pred, target):
    """Full numpy fallback with reference semantics (degenerate cases only)."""
    b, c, h, w = pred.shape
    n = b * h * w
    t = target.reshape(-1).astype(np.int64)
    valid = t != 255
    t0 = np.where(valid, t, 0)
    logits = np.transpose(pred, (0, 2, 3, 1)).reshape(n, c).astype(np.float32)
    m = logits.max(axis=1, keepdims=True)
    ex = np.exp(logits - m)
    s = ex.sum(axis=1)
    pt = ex[np.arange(n), t0] / s
    mask_prob = np.where(valid, pt, 1.0).astype(np.float32)
    kth = np.sort(mask_prob)[min(n, MIN_KEPT) - 1]
    thr = max(float(kth), THRESH)
    kept = mask_prob <= thr
    fv = valid & kept
    nll = (np.log(s) + m[:, 0] - logits[np.arange(n), t0]).astype(np.float32)
    num = float(np.where(fv, nll, 0.0).sum(dtype=np.float64))
    cnt = float(fv.sum())
    return np.float32(num / max(cnt, 1.0))


def _run_device(in_maps, g, trace=False):
    nc = _get_nc(g)
    return run_bass_kernel_spmd(nc, in_maps, list(range(NCORES)), trace=trace)


def kernel(pred, target):
    pred = np.asarray(pred)
    target = np.asarray(target)
    assert pred.shape == (BATCH, C, HH, WW), pred.shape
    assert target.shape == (BATCH, HH, WW), target.shape

    if target.min() < 0 or target.max() >= C:
        # ignore_index / out-of-range labels: not producible by the input
        # spec (randint 0..18); handle via the host reference for safety.
        return _reference_numpy(pred, target)

    g = GEO_FULL
    in_maps = _shard_inputs(pred, target, g)
    res = _run_device(in_maps, g).results

    num = 0.0
    cnt = 0.0
    for core in range(NCORES):
        a = res[core]["acc"].astype(np.float64)
        num += a[0, :512].sum()
        cnt += a[0, 512:].sum()
    num += NLL_MIN * cnt

    if cnt < MIN_KEPT:
        # kth-smallest prob exceeds 0.7: threshold is data-dependent.
        return _reference_numpy(pred, target)

    return np.float32(num / max(cnt, 1.0))


# revision 63
# speedup vs baseline: 1.0166x; 1.0009x over previous
"""OHEM cross-entropy loss (CriterionOhem) on 8 Trainium2 NeuronCores.

Reference semantics (N = 4*512*1024 pixels, C = 19 classes):
  p_i     = softmax(pred)[i, t_i]                (true-class prob per pixel)
  kth     = sort(p)[MIN_KEPT-1]
  thr     = max(kth, 0.7)
  keep_i  = p_i <= thr
  loss    = sum(keep_i * nll_i) / max(count(keep), 1)

Key reduction: if count(p <= 0.7) >= MIN_KEPT then kth <= 0.7 and thr == 0.7
exactly, so no top-k is needed — only a masked sum + count, which the host
verifies from the device partials (falling back to a numpy reference in the
degenerate case, which cannot occur for randn logits).

Host-side trick: per pixel, the target-class logit is swapped into class
slot 0 (a pure permutation — the softmax denominator is permutation
invariant and x_t becomes the slot-0 plane).  This removes every trace of
the target from the device program: no replicated (t - c) bytes on the
wire, no masked-logit pass on DVE, no second matmul pass on PE.

Device layout per core (262,144 pixels = 128 blocks x 2048):
  xk   [128*19, 2048] fp8    class-rows, row g = 19*block + class
  xk16 [3*128, 2048]  fp16   the three DVE_TILES' rows (DVE needs 2-byte)
  x0   [128, 2048]    fp16   slot-0 (target-class) logits + NLL_MIN
  19 tiles of 128 consecutive class-rows stream through:
    ACT: et = exp(x) for 16 tiles (chunked to amortize fixed overhead);
    DVE: et = exp(x) for 3 tiles via a bit-trick exp2 (engine balancing);
    PE : one-hot block maps contract the 19 classes of each block into
         S[block, pixel], accumulated across all 19 tiles into four
         [128, 512] fp32 PSUM banks (start@k=0 / stop@k=18).
  Finals per bank as its accumulation group stops: lnS = ln(S) on ACT;
  d1 = lnS - x0' on DVE; rl = relu(d1), kp = (d1 >= 0); PE ones-matmuls
  fold partitions into R[1, 512] accumulators across banks; host sums
  num = sum(rl) + NLL_MIN*cnt, cnt = sum(kp).
"""

import numpy as np
from contextlib import ExitStack

import concourse.bass as bass
import concourse.tile as tile
from concourse import bacc
from concourse import mybir
from concourse.bass_utils import run_bass_kernel_spmd

F32 = mybir.dt.float32
F16 = mybir.dt.float16
F8 = mybir.dt.float8e4
AF = mybir.ActivationFunctionType
OP = mybir.AluOpType

C = 19
THRESH = 0.7
MIN_KEPT = 100000
NLL_MIN = float(-np.log(np.float32(THRESH)))  # keep <=> nll >= -ln(0.7)

# Full-size geometry: 4x19x512x1024 pred over 8 cores.
BATCH, HH, WW = 4, 512, 1024
NCORES = 8


class Geo:
    def __init__(self, X, nblk):
        self.X = X                      # pixels per block (free axis)
        self.NBLK = nblk                # blocks per core (psum partitions)
        self.NROWS = C * nblk           # class-rows per core
        self.NT = self.NROWS // 128     # 128-row tiles
        assert self.NROWS % 128 == 0
        self.NPIX = nblk * X            # pixels per core


GEO_FULL = Geo(2048, 128)               # 128 x 2048 = 262,144 pixels/core


def make_consts(g):
    """Per-tile one-hot block maps, packed side by side: map k column block
    [k*128,(k+1)*128) has a one at (p, (128k+p)//19)."""
    maps = np.zeros((128, g.NT * 128), np.float16)
    for k in range(g.NT):
        for p in range(128):
            maps[p, k * 128 + (128 * k + p) // C] = 1.0
    return {"maps": maps}


ACT_TABLE_LN_EXP = 6  # natural_log_exp_and_others in act_info.json

# Tiles whose exp runs on DVE (bit-trick exp2) instead of ACT, balancing the
# two engines; spread through the stream so matmul feeding stays smooth.
DVE_TILES = (4, 9, 14)
# Minimax quadratic for 2^f on [-0.5, 0.5]: 2^f ~= S2*(f+A2)^2 + C2.
S2, A2, C2 = 0.22266791031510733, 1.556176036576733, 0.4609102972256174
L2E = float(np.log2(np.e))


def _chunk_plan(g):
    """Shared ACT chunking rule: leading single, pairs, three trailing
    singles (their matmuls hide under the following exps)."""
    act = [k for k in range(g.NT) if k not in DVE_TILES]
    pairs = []
    i = 1
    while i + 3 < len(act):
        pairs.append((act[i], act[i + 1]))
        i += 2
    singles = [act[0]] + act[i:]
    return pairs, singles


def emit(ctx, tc, g, xk, xks, xk16, x0, maps, acc):
    nc = tc.nc
    X = g.X
    NSLC = X // 512
    I16 = mybir.dt.int16

    # ACT work items: chunks of tiles sharing one SBUF tile / ACT exp.
    # A leading single (fast first exp) and three trailing singles (each
    # tile's matmuls hide under the next tile's exp, shrinking the PE
    # drain that gates the finals); pairs in the middle amortize the
    # fixed per-instruction overhead.  DVE tiles are woven in between.
    act_tiles = [k for k in range(g.NT) if k not in DVE_TILES]
    chunks = [(act_tiles[0],)]
    i = 1
    while i + 3 < len(act_tiles):
        chunks.append((act_tiles[i], act_tiles[i + 1]))
        i += 2
    chunks += [(k,) for k in act_tiles[i:]]
    # DVE exp work is woven in early (DVE is otherwise idle until the
    # finals); its matmuls are deferred to the second-to-last item so PE
    # digests them under the trailing exps (PE executes in order, so a
    # DVE-gated matmul mid-stream would block later ACT-tile matmuls).
    pair_idx = {ch: n for n, ch in
                enumerate(c for c in chunks if len(c) == 2)}
    single_idx = {c[0]: n for n, c in
                  enumerate(c for c in chunks if len(c) == 1)}
    items = []
    for ci, ch in enumerate(chunks):
        items.append(("act", ch))
        if 0 < ci <= len(DVE_TILES):
            items.append(("dve", ci - 1))

    xvp = ctx.enter_context(tc.tile_pool(name="xvp", bufs=4))
    etp = ctx.enter_context(tc.tile_pool(name="etp", bufs=4))
    cst = ctx.enter_context(tc.tile_pool(name="cst", bufs=1))
    pss = ctx.enter_context(tc.tile_pool(name="pss", bufs=1, space="PSUM"))
    fin = ctx.enter_context(tc.tile_pool(name="fin", bufs=1))
    lnp = ctx.enter_context(tc.tile_pool(name="lnp", bufs=4))
    rkp = ctx.enter_context(tc.tile_pool(name="rkp", bufs=4))
    dxp = ctx.enter_context(tc.tile_pool(name="dxp", bufs=2))
    dep = ctx.enter_context(tc.tile_pool(name="dep", bufs=len(DVE_TILES)))
    dsp = ctx.enter_context(tc.tile_pool(name="dsp", bufs=2))

    # One activation table serves both Exp and Ln; load it up front so the
    # compiler's table-load pass inserts nothing mid-stream.
    nc.scalar.add_instruction(mybir.InstLoadActFuncSet(
        name=nc.get_next_instruction_name(), ins=[], outs=[],
        act_func_set_id=ACT_TABLE_LN_EXP))

    maps_t = cst.tile([128, g.NT * 128], F16)
    x0t = cst.tile([128, X], F16)
    ones_t = cst.tile([128, 1], F16)
    nc.vector.memset(ones_t[:], 1.0)

    # One PSUM tile per 512-column bank: a PSUM reader waits for its whole
    # accumulation group, so separate tiles let each bank's finals start as
    # soon as that bank's own 19-matmul group stops.
    S_ps = [pss.tile([128, 512], F32, name=f"S{m}") for m in range(NSLC)]
    # Final partial sums: R[0,:] accumulates relu(d1) column sums, R[1,:]
    # keep-count column sums, via ones-vector matmuls over the 4 banks.
    R_rl = pss.tile([1, 512], F32)
    R_kp = pss.tile([1, 512], F32)

    def mm_tile(k, et, off, m0=0, m1=NSLC):
        lhs = maps_t[:, k * 128:(k + 1) * 128]
        for m in range(m0, m1):
            nc.tensor.matmul(S_ps[m][:, :], lhs,
                             et[:, off + m * 512:off + (m + 1) * 512],
                             start=(k == 0), stop=(k == g.NT - 1))

    def dma_item(i):
        kind, v = items[i]
        if kind == "act":
            xv = xvp.tile([128, len(v) * X], F8, tag="x")
            if len(v) == 2:
                pi = pair_idx[v]
                nc.sync.dma_start(xv[:], xk[128 * pi:128 * (pi + 1), :])
            else:
                si = single_idx[v[0]]
                nc.sync.dma_start(xv[:], xks[128 * si:128 * (si + 1), :])
        else:
            xv = dxp.tile([128, X], F16, tag="dx")
            nc.sync.dma_start(xv[:], xk16[128 * v:128 * (v + 1), :])
        # Map loads: small per-chunk slices early (per-DMA HWDGE setup is
        # ~625ns, but a single bulk load early would displace the pixel
        # tiles that gate ACT), then one bulk DMA for tiles 8+ once the
        # queue has slack.  Each lands before the matmuls that read it.
        if i == 0:
            nc.sync.dma_start(maps_t[:, 0:128], maps[:, 0:128])
        elif i == 1:
            nc.sync.dma_start(maps_t[:, 128:384], maps[:, 128:384])
        elif i == 3:
            nc.sync.dma_start(maps_t[:, 384:768], maps[:, 384:768])
        elif i == 5:
            nc.sync.dma_start(maps_t[:, 768:1024], maps[:, 768:1024])
        elif i == 6:
            nc.sync.dma_start(maps_t[:, 1024:], maps[:, 1024:])
        if i == len(items) - 2:
            nc.sync.dma_start(x0t[:], x0)
        return xv

    def dve_exp(xv, et):
        """et = exp(xv) on DVE: 2^(x*log2 e) via the fp16 +1536 rounding
        trick for the integer part (exponent-field construction) and a
        minimax quadratic for 2^frac.  ~0.9% max rel error."""
        s1 = dsp.tile([128, X], F16, tag="s1")
        s2 = dsp.tile([128, X], F16, tag="s2")
        s3 = dsp.tile([128, X], F16, tag="s3")
        TS, TT = nc.vector.tensor_scalar, nc.vector.tensor_tensor
        TS(out=s1[:], in0=xv[:], scalar1=L2E, scalar2=1536.0,
           op0=OP.mult, op1=OP.add)                      # s1 = 1536 + n
        TS(out=s2[:], in0=s1[:], scalar1=-1536.0, scalar2=None,
           op0=OP.add)                                   # s2 = n
        TS(out=s3[:], in0=xv[:], scalar1=L2E, scalar2=None,
           op0=OP.mult)                                  # s3 = y
        TT(out=s3[:], in0=s3[:], in1=s2[:], op=OP.subtract)  # s3 = f
        TS(out=s1[:].bitcast(I16), in0=s1[:].bitcast(I16),
           scalar1=15 - 0x6600, scalar2=None, op0=OP.add)
        TS(out=s1[:].bitcast(I16), in0=s1[:].bitcast(I16),
           scalar1=10, scalar2=None, op0=OP.logical_shift_left)  # s1 = 2^n
        TS(out=s2[:], in0=s3[:], scalar1=float(A2), scalar2=None,
           op0=OP.add)                                   # s2 = f + A2
        TT(out=s2[:], in0=s2[:], in1=s2[:], op=OP.mult)  # s2 = (f+A2)^2
        TS(out=s2[:], in0=s2[:], scalar1=float(S2), scalar2=float(C2),
           op0=OP.mult, op1=OP.add)                      # s2 ~= 2^f
        TT(out=et[:], in0=s2[:], in1=s1[:], op=OP.mult)  # et = 2^f * 2^n

    # Software-pipelined: item i+1's DMAs are emitted (and queued) before
    # item i's compute so the load stream never waits on compute emission.
    dve_ets = []
    xv_next = dma_item(0)
    for i, (kind, v) in enumerate(items):
        xv = xv_next
        if i + 1 < len(items):
            xv_next = dma_item(i + 1)
        if kind == "dve":
            et = dep.tile([128, X], F16, tag="de")
            dve_exp(xv, et)
            dve_ets.append(et)
            continue
        if i == len(items) - 2:
            # Deferred DVE-tile matmuls (their ets completed mid-stream):
            # emitted here so PE digests them under the second-to-last
            # exp, leaving only the last tile's matmuls trailing.
            for m in range(NSLC):
                for di, det in enumerate(dve_ets):
                    mm_tile(DVE_TILES[di], det, 0, m, m + 1)
        W = len(v) * X
        et = etp.tile([128, W], F16, tag="e")
        # The last tile's exp runs in halves so its matmuls (which gate the
        # finals) start two PSUM banks early.
        hsplit = 2 if i == len(items) - 1 else 1
        for h in range(hsplit):
            hw = W // hsplit
            nc.scalar.activation(et[:, h * hw:(h + 1) * hw],
                                 xv[:, h * hw:(h + 1) * hw], AF.Exp)
            for j, k in enumerate(v):
                mm_tile(k, et, j * X, h * NSLC // hsplit,
                        (h + 1) * NSLC // hsplit)

    # Per-bank finals (x0 already holds x_t + NLL_MIN from the host):
    #   d1 = ln S - x0' ; rl = relu(d1) ; kp = (d1 >= 0)
    # then PE ones-matmuls fold the partition dimension into R_{rl,kp},
    # accumulating across banks; the host sums the 512-wide rows.
    for m in range(NSLC):
        sl = slice(m * 512, (m + 1) * 512)
        lnS = lnp.tile([128, 512], F16, tag="ln")
        nc.scalar.activation(lnS[:], S_ps[m][:, :], AF.Ln)
        rk = rkp.tile([128, 3, 512], F16, tag="rk")
        d1 = rk[:, 2, :]
        nc.vector.tensor_sub(d1, lnS[:], x0t[:, sl])
        nc.vector.tensor_scalar(out=rk[:, 0, :], in0=d1, scalar1=0.0,
                                scalar2=None, op0=OP.max)
        nc.vector.tensor_scalar(out=rk[:, 1, :], in0=d1, scalar1=0.0,
                                scalar2=None, op0=OP.is_ge)
        nc.tensor.matmul(R_rl[:, :], ones_t[:], rk[:, 0, :],
                         start=(m == 0), stop=(m == NSLC - 1))
        nc.tensor.matmul(R_kp[:, :], ones_t[:], rk[:, 1, :],
                         start=(m == 0), stop=(m == NSLC - 1))

    # Evacuate the two PSUM rows in parallel on ACT and DVE (both engines
    # must address partition 0 — engine partition bases are 0/32/64/96),
    # then one DMA.
    accs = fin.tile([1, 1024], F32)
    nc.vector.tensor_scalar(out=accs[0:1, 0:512], in0=R_rl[:, :],
                            scalar1=0.0, scalar2=None, op0=OP.add)
    nc.scalar.copy(accs[0:1, 512:1024], R_kp[:, :])
    nc.sync.dma_start(acc[:, :], accs[:])


def build_nc(g):
    nc = bacc.Bacc(
        "TRN2",
        target_bir_lowering=False,
        debug=False,
        enable_asserts=True,
        num_devices=NCORES,
    )
    pairs, singles = _chunk_plan(g)
    xk = nc.dram_tensor("xk", [len(pairs) * 128, 2 * g.X], F8,
                        kind="ExternalInput")
    xks = nc.dram_tensor("xks", [len(singles) * 128, g.X], F8,
                         kind="ExternalInput")
    xk16 = nc.dram_tensor("xk16", [len(DVE_TILES) * 128, g.X], F16,
                          kind="ExternalInput")
    x0 = nc.dram_tensor("x0", [g.NBLK, g.X], F16, kind="ExternalInput")
    maps = nc.dram_tensor("maps", [128, g.NT * 128], F16, kind="ExternalInput")
    acc = nc.dram_tensor("acc", [1, 1024], F32, kind="ExternalOutput")
    with tile.TileContext(nc) as tc, ExitStack() as ctx:
        emit(ctx, tc, g, xk.ap(), xks.ap(), xk16.ap(), x0.ap(), maps.ap(),
             acc.ap())
    nc.compile()
    return nc


_NC_CACHE = {}


def _get_nc(g):
    key = (g.X, g.NBLK)
    if key not in _NC_CACHE:
        _NC_CACHE[key] = build_nc(g)
    return _NC_CACHE[key]


def make_inputs(pred_slice, target_slice, g):
    """Per-core packed inputs.  xk[b*19 + c, px] holds the fp8 logits with
    the target class swapped into slot 0; x0 is the slot-0 (target-class)
    fp16 plane with the keep threshold pre-added (device computes
    d1 = ln S - x0')."""
    import ml_dtypes
    xk = np.empty((g.NBLK, C, g.X), np.float16)
    xk[:] = pred_slice.reshape(C, g.NBLK, g.X).swapaxes(0, 1)
    t = target_slice.reshape(g.NBLK, g.X).astype(np.intp)
    bi = np.arange(g.NBLK)[:, None]
    ci = np.arange(g.X)[None, :]
    xt = xk[bi, t, ci].copy()
    xk[bi, t, ci] = xk[:, 0, :]
    xk[:, 0, :] = xt
    xkr = xk.reshape(g.NROWS, g.X)
    xk8 = xkr.astype(ml_dtypes.float8_e4m3)
    pairs, singles = _chunk_plan(g)
    xkp = np.concatenate(
        [np.concatenate([xk8[128 * a:128 * (a + 1), :],
                         xk8[128 * b:128 * (b + 1), :]], axis=1)
         for a, b in pairs])
    xksg = np.concatenate([xk8[128 * k:128 * (k + 1), :]
                           for k in singles])
    xk16 = np.concatenate([xkr[128 * k:128 * (k + 1), :] for k in DVE_TILES])
    x0 = (xt.astype(np.float32) + np.float32(NLL_MIN)).astype(np.float16)
    return {"xk": xkp, "xks": xksg, "xk16": xk16, "x0": x0}


def _shard_inputs(pred, target, g):
    """Slice the full inputs into per-core in_maps (8 cores)."""
    consts = make_consts(g)
    in_maps = []
    rows_per_core = HH // 2  # 256
    for core in range(NCORES):
        b, half = core // 2, core % 2
        h0 = half * rows_per_core
        m = make_inputs(pred[b, :, h0:h0 + rows_per_core, :],
                        target[b, h0:h0 + rows_per_core, :], g)
        m.update(consts)
        in_maps.append(m)
    return in_maps


def _reference_numpy(pred, target):
    """Full numpy fallback with reference semantics (degenerate cases only)."""
    b, c, h, w = pred.shape
    n = b * h * w
    t = target.reshape(-1).astype(np.int64)
    valid = t != 255
    t0 = np.where(valid, t, 0)
    logits = np.transpose(pred, (0, 2, 3, 1)).reshape(n, c).astype(np.float32)
    m = logits.max(axis=1, keepdims=True)
    ex = np.exp(logits - m)
    s = ex.sum(axis=1)
    pt = ex[np.arange(n), t0] / s
    mask_prob = np.where(valid, pt, 1.0).astype(np.float32)
    kth = np.sort(mask_prob)[min(n, MIN_KEPT) - 1]
    thr = max(float(kth), THRESH)
    kept = mask_prob <= thr
    fv = valid & kept
    nll = (np.log(s) + m[:, 0] - logits[np.arange(n), t0]).astype(np.float32)
    num = float(np.where(fv, nll, 0.0).sum(dtype=np.float64))
    cnt = float(fv.sum())
    return np.float32(num / max(cnt, 1.0))


def _run_device(in_maps, g, trace=False):
    nc = _get_nc(g)
    return run_bass_kernel_spmd(nc, in_maps, list(range(NCORES)), trace=trace)


def kernel(pred, target):
    pred = np.asarray(pred)
    target = np.asarray(target)
    assert pred.shape == (BATCH, C, HH, WW), pred.shape
    assert target.shape == (BATCH, HH, WW), target.shape

    if target.min() < 0 or target.max() >= C:
        # ignore_index / out-of-range labels: not producible by the input
        # spec (randint 0..18); handle via the host reference for safety.
        return _reference_numpy(pred, target)

    g = GEO_FULL
    in_maps = _shard_inputs(pred, target, g)
    res = _run_device(in_maps, g).results

    num = 0.0
    cnt = 0.0
    for core in range(NCORES):
        a = res[core]["acc"].astype(np.float64)
        num += a[0, :512].sum()
        cnt += a[0, 512:].sum()
    num += NLL_MIN * cnt

    if cnt < MIN_KEPT:
        # kth-smallest prob exceeds 0.7: threshold is data-dependent.
        return _reference_numpy(pred, target)

    return np.float32(num / max(cnt, 1.0))
